# revision 1
# baseline (speedup 1.0000x reference)
"""Multi-head attention (16 heads, d_model=1024, S=2048) on 8 Trainium2 cores.

Sharding: tensor-parallel over heads — each core owns 2 heads (its slice of
Wq/Wk/Wv and the matching 128 columns of Q/K/V and of Wo).  Each core computes
its heads' attention and a row-parallel partial of the final linear; the host
sums the 8 partials and adds bo (the unshard step for row-parallel).

Device-side layout ("T-space"): activations are kept transposed, [feature,
seq], so that
  - projections contract the input feature dim (on partitions),
  - scores^T[t, s] = k_chunk.T @ qT needs no transpose of the attention matrix,
  - softmax denominators come free via a ones-column appended to V,
  - attn@V and the final linear consume exp(scores^T) chunks directly as the
    moving operand at full PE rate (f32r, N=512).
Only the raw Q/K/V input slices (and V again, post-projection, with the ones
row) are transposed, via cheap 128-wide PE transposes.  Softmax is computed
without max-subtraction: scores here are bounded (|s| < ~10), far from fp32
exp overflow, and softmax is shift-invariant.

Matmul operands use float32r (full-rate fp32 streaming, TF32-grade rounding,
~1e-4 relative per matmul).  Set MHA_PREC=f32 for exact-fp32 matmuls
(4x slower on the PE).
"""

import os
import sys

for _p in ("/opt/trn_rl_repo",):
    if _p not in sys.path:
        sys.path.insert(0, _p)

from contextlib import ExitStack

import numpy as np

import concourse.bass as bass
import concourse.tile as tile
from concourse import bacc, mybir
from concourse.bass import ts
from concourse.bass_utils import run_bass_kernel_spmd
from concourse.masks import make_identity

S = 2048          # sequence length
DK = 1024         # d_model
H = 16            # heads
DH = 64           # head dim
NCORES = 8
CW = 128          # per-core feature slice width (2 heads x 64)
NCH = S // 128    # 16 chunks of 128 along the sequence

F32 = mybir.dt.float32
F32R = mybir.dt.float32r
EXP = mybir.ActivationFunctionType.Exp

_CACHE = {}


def _build_nc(rdt):
    nc = bacc.Bacc(
        "TRN2", target_bir_lowering=False, debug=False, enable_asserts=False
    )

    def dma_cast(dst_ap, src_ap):
        # pure bitcast on the DRAM side when loading into f32r tiles
        nc.sync.dma_start(dst_ap, src_ap.bitcast(rdt) if rdt != F32 else src_ap)

    Qs = nc.dram_tensor("Qs", [S, CW], F32, kind="ExternalInput")
    Ks = nc.dram_tensor("Ks", [S, CW], F32, kind="ExternalInput")
    Vs = nc.dram_tensor("Vs", [S, CW], F32, kind="ExternalInput")
    Wtq = nc.dram_tensor("Wtq", [CW, CW], F32, kind="ExternalInput")
    Wtk = nc.dram_tensor("Wtk", [CW, CW], F32, kind="ExternalInput")
    Wtv = nc.dram_tensor("Wtv", [CW, CW], F32, kind="ExternalInput")
    Bq = nc.dram_tensor("Bq", [CW, 1], F32, kind="ExternalInput")
    Bk = nc.dram_tensor("Bk", [CW, 1], F32, kind="ExternalInput")
    Bv0 = nc.dram_tensor("Bv0", [DH, 1], F32, kind="ExternalInput")
    Bv1 = nc.dram_tensor("Bv1", [DH, 1], F32, kind="ExternalInput")
    WoT = nc.dram_tensor("WoT", [CW, DK], F32, kind="ExternalInput")
    WoT1 = nc.dram_tensor("WoT1", [DH, DK], F32, kind="ExternalInput")
    PT = nc.dram_tensor("PT", [DK, S], F32, kind="ExternalOutput")
    # head-1 contribution to the last s-half, unnormalized (the host scales
    # its columns by 1/DN during the cross-core reduce) - this keeps the
    # final softmax-normalize chain off the kernel's critical path
    PTB = nc.dram_tensor("PTB", [DK, S // 2], F32, kind="ExternalOutput")
    DN = nc.dram_tensor("DN", [1, S // 2], F32, kind="ExternalOutput")

    with tile.TileContext(nc) as tc:
        with ExitStack() as ctx:
            pers = ctx.enter_context(tc.tile_pool(name="pers", bufs=1))
            expool = ctx.enter_context(tc.tile_pool(name="expool", bufs=3))
            stage = ctx.enter_context(tc.tile_pool(name="stage", bufs=4))
            nrm = ctx.enter_context(tc.tile_pool(name="nrm", bufs=2))
            psmm = ctx.enter_context(tc.tile_pool(name="psmm", bufs=2, space="PSUM"))
            psacc = ctx.enter_context(tc.tile_pool(name="psacc", bufs=2, space="PSUM"))
            dscr = ctx.enter_context(tc.tile_pool(name="dscr", bufs=2, space="DRAM"))

            # ---- load raw activation slices, chunk-major (issued first so
            # the transposes can start as early as possible) ----
            # raw[:, j*128:(j+1)*128] = X[j*128:(j+1)*128, :]
            qraw = pers.tile([128, S], rdt, tag="qraw")
            kraw = pers.tile([128, S], rdt, tag="kraw")
            vraw = pers.tile([128, S], rdt, tag="vraw")
            for raw, dram in ((qraw, Qs), (kraw, Ks), (vraw, Vs)):
                src = dram.ap().bitcast(rdt) if rdt != F32 else dram.ap()
                src = src.rearrange("(q j p) d -> q p j d", q=4, p=128)
                dst = raw[:].rearrange("p (q j d) -> q p j d", q=4, d=CW)
                for q in range(4):  # quarters, so transposes can start early
                    nc.sync.dma_start(dst[q], src[q])

            # ---- constants / weights ----
            ident = pers.tile([128, 128], F32, tag="ident")
            make_identity(nc, ident[:])
            if rdt == F32:
                ident_r = ident
            else:
                ident_r = pers.tile([128, 128], rdt, tag="identr")
                nc.vector.tensor_copy(ident_r[:], ident[:])

            wtq_sb = pers.tile([CW, CW], rdt, tag="wtq")
            wtk_sb = pers.tile([CW, CW], rdt, tag="wtk")
            wtv_sb = pers.tile([CW, CW], rdt, tag="wtv")
            dma_cast(wtq_sb[:], Wtq.ap())
            dma_cast(wtk_sb[:], Wtk.ap())
            dma_cast(wtv_sb[:], Wtv.ap())

            bq_sb = pers.tile([CW, 1], F32, tag="bq")
            bk_sb = pers.tile([CW, 1], F32, tag="bk")
            bv0_sb = pers.tile([DH, 1], F32, tag="bv0")
            bv1_sb = pers.tile([DH, 1], F32, tag="bv1")
            nc.sync.dma_start(bq_sb[:], Bq.ap())
            nc.sync.dma_start(bk_sb[:], Bk.ap())
            nc.sync.dma_start(bv0_sb[:], Bv0.ap())
            nc.sync.dma_start(bv1_sb[:], Bv1.ap())

            wo_sb = pers.tile([CW, DK], rdt, tag="wo")
            dma_cast(wo_sb[:], WoT.ap())
            wo1_sb = pers.tile([DH, DK], rdt, tag="wo1")
            dma_cast(wo1_sb[:], WoT1.ap())

            # ---- PE-transpose raw chunks into T-space: XT[d2, s] ----
            # (f32r transpose mode: 1.5 cyc/row vs 2 for fp32)
            QT = pers.tile([128, S], rdt, tag="QT")
            KT = pers.tile([128, S], rdt, tag="KT")
            VT = pers.tile([128, S], rdt, tag="VT")
            for raw, xt in ((qraw, QT), (kraw, KT), (vraw, VT)):
                for j in range(NCH):
                    pt = psmm.tile([128, 128], rdt, tag="ps")
                    nc.tensor.transpose(pt[:], raw[:, ts(j, 128)], ident_r[:])
                    # split the drain copies across both engines - a single
                    # serial DVE copy chain gates the whole prologue
                    if j % 2 == 0:
                        nc.vector.tensor_copy(xt[:, ts(j, 128)], pt[:])
                    else:
                        nc.scalar.copy(xt[:, ts(j, 128)], pt[:])

            # ---- projections (T-space): xT = blockdiag(W.T) @ XT + b ----
            # k lands in per-head tiles with the other head's rows zeroed, so
            # the scores matmuls can use K=128 stationaries (full PE-array
            # activity keeps the HAM clock at 2.4 GHz; K=64 streams count as
            # ~50% activity and the PE decays to 1.2 GHz).
            qTs = pers.tile([128, S], rdt, tag="qTs")
            kp0 = pers.tile([128, S], rdt, tag="kp0")
            kp1 = pers.tile([128, S], rdt, tag="kp1")
            kp = [kp0, kp1]
            nc.gpsimd.memset(kp[0][:].bitcast(F32), 0.0)
            nc.gpsimd.memset(kp[1][:].bitcast(F32), 0.0)
            IDENT_FN = mybir.ActivationFunctionType.Identity
            for sl in range(S // 512):
                pp = psmm.tile([128, 512], F32, tag="ps")
                nc.tensor.matmul(pp[:], wtq_sb[:], QT[:, ts(sl, 512)])
                # bias-add on ACT (out = 1.0*in + b) to keep DVE free
                nc.scalar.activation(
                    qTs[:, ts(sl, 512)], pp[:], IDENT_FN, bias=bq_sb[:]
                )
            for sl in range(S // 512):
                pp = psmm.tile([128, 512], F32, tag="ps")
                nc.tensor.matmul(pp[:], wtk_sb[:], KT[:, ts(sl, 512)])
                nc.vector.tensor_scalar_add(
                    kp[0][0:DH, ts(sl, 512)], pp[0:DH, :], bk_sb[0:DH]
                )
                nc.vector.tensor_scalar_add(
                    kp[1][DH:128, ts(sl, 512)], pp[DH:128, :], bk_sb[DH:128]
                )

            # v per head, with a ones row appended (softmax denominator trick)
            vaug = []
            for h, bvh in ((0, bv0_sb), (1, bv1_sb)):
                va = pers.tile([DH + 1, S], F32, tag=f"vaug{h}")
                nc.gpsimd.memset(va[DH : DH + 1, :], 1.0)
                for sl in range(S // 512):
                    pp = psmm.tile([DH, 512], F32, tag="ps")
                    nc.tensor.matmul(
                        pp[:], wtv_sb[:, ts(h, DH)], VT[:, ts(sl, 512)]
                    )
                    nc.vector.tensor_scalar_add(va[0:DH, ts(sl, 512)], pp[:], bvh[:])
                vaug.append(va)

            # ---- transpose v back to [t, e|1] chunks (stationary for attn@V) ----
            # chunks padded to 128 columns of which 65..127 stay zero: the
            # attn@V matmuls then use full-array [128,128] stationaries
            # (HAM warmth; the zero columns just write zeros to psum rows
            # 65..127, which are never read).
            vS = []
            for h in (0, 1):
                vs = pers.tile([128, NCH * 128], rdt, tag=f"vS{h}")
                nc.gpsimd.memset(vs[:].bitcast(F32), 0.0)
                for j in range(NCH):
                    pt = psmm.tile([128, DH + 1], F32, tag="ps")
                    nc.tensor.transpose(
                        pt[:],
                        vaug[h][:, ts(j, 128)],
                        ident[0 : DH + 1, 0 : DH + 1],
                    )
                    if j % 2 == 0:
                        nc.vector.tensor_copy(
                            vs[:, j * 128 : j * 128 + DH + 1], pt[:]
                        )
                    else:
                        nc.scalar.copy(vs[:, j * 128 : j * 128 + DH + 1], pt[:])
                vS.append(vs)

            # ---- attention, per head, per s-half ----
            oT_all = pers.tile([128, S], rdt, tag="oT")

            def attention_round(h, sh, extras=(), extras_from=6):
                """One (head, s-half) round, software-pipelined: MM3' for
                chunk j-1 is emitted after MM2' of chunk j, so the in-order
                PE queue never stalls waiting for the exp of the current
                chunk.  `extras` are final-linear quarter closures woven one
                per chunk (from chunk `extras_from`) into the PE stream -
                they ride in the ACT-gated slack and keep the PE from ever
                idling (and its HAM clock from dropping to 1.2 GHz)."""
                hs = h * DH
                s0 = sh * 1024
                acc = psacc.tile([128, 1024], F32, tag="acc")
                exs = [None] * NCH
                extras = list(extras)

                def mm2(j):
                    sc = psmm.tile([128, 1024], F32, tag="ps")
                    for n in range(2):
                        nc.tensor.matmul(
                            sc[:, ts(n, 512)],
                            kp[h][:, ts(j, 128)],
                            qTs[:, s0 + n * 512 : s0 + (n + 1) * 512],
                        )
                    ex = expool.tile([128, 1024], rdt, tag="ex")
                    nc.scalar.activation(ex[:], sc[:], EXP, scale=0.125)
                    exs[j] = ex

                def mm3(j):
                    for n in range(2):
                        nc.tensor.matmul(
                            acc[:, ts(n, 512)],
                            vS[h][:, ts(j, 128)],
                            exs[j][:, ts(n, 512)],
                            start=(j == 0),
                            stop=(j == NCH - 1),
                        )

                mm2(0)
                for j in range(1, NCH):
                    mm2(j)
                    mm3(j - 1)
                    if j >= extras_from and extras:
                        extras.pop(0)()
                mm3(NCH - 1)
                while extras:
                    extras.pop(0)()

                # rows 0..63 are unnormalized o^T, row 64 the softmax denom
                oc = nrm.tile([DH + 1, 1024], rdt, tag="oc")
                nc.vector.tensor_copy(oc[:], acc[0 : DH + 1, :])
                if h == 1 and sh == 1:
                    # last round: skip the on-device normalize entirely - the
                    # unnormalized o^T goes straight into the PTB partial and
                    # the host divides by the denominator during the reduce
                    nc.sync.dma_start(DN.ap(), oc[DH : DH + 1, :].bitcast(F32))
                    return oc
                # The denominator row is bounced through DRAM twice: once
                # reshaped to [64, 16] so the reciprocal uses all lanes (DVE
                # reciprocal is ~6.4 ns/elem/lane), once partition-broadcast
                # to [64, 1024] for the normalize multiply.  All off the PE
                # queue; overlapped by the next round's attention.
                dnd = dscr.tile([1, 1024], F32, tag="dnd")
                nc.sync.dma_start(dnd[:], oc[DH : DH + 1, :].bitcast(F32))
                d16 = nrm.tile([DH, 16], F32, tag="d16")
                nc.sync.dma_start(
                    d16[:], dnd[0:1, :].rearrange("a (p f) -> (a p) f", p=DH)
                )
                r16 = nrm.tile([DH, 16], F32, tag="r16")
                nc.vector.reciprocal(r16[:], d16[:])
                rnd = dscr.tile([1, 1024], F32, tag="rnd")
                nc.sync.dma_start(
                    rnd[0:1, :].rearrange("a (p f) -> (a p) f", p=DH), r16[:]
                )
                rb = nrm.tile([DH, 1024], F32, tag="rb")
                nc.sync.dma_start(rb[:], rnd[0:1, :].to_broadcast((DH, 1024)))
                ot = nrm.tile([DH, 1024], rdt, tag="ot")
                nc.vector.tensor_mul(ot[:], oc[0:DH, :].bitcast(F32), rb[:])
                nc.sync.dma_start(oT_all[hs : hs + DH, s0 : s0 + 1024], ot[:])
                return None

            def fl_quarter(lhsT, rhs, out_slice, use_act=False, tag="acc"):
                """One [128, 512] quarter of a final-linear partial:
                matmul -> stage copy -> DMA out.  Inside a round, quarters
                use the spare psacc slot (scores own both psmm slots);
                in the tail, callers alternate pools explicitly."""

                def emit():
                    p = (psacc if tag == "acc" else psmm).tile(
                        [128, 512], F32, tag=tag
                    )
                    nc.tensor.matmul(p[:], lhsT, rhs)
                    st = stage.tile([128, 512], F32, tag="st")
                    if use_act:
                        nc.scalar.copy(st[:], p[:])
                    else:
                        nc.vector.tensor_copy(st[:], p[:])
                    nc.sync.dma_start(out_slice, st[:])

                return emit

            # final-linear quarter lists.
            # fl_a: first s-half, K=128 both heads (needs sh=0 normalizes,
            #       complete ~6us into round (0,1)) - woven into (0,1)/(1,1)
            # fl_b: second s-half, head-0 K=64 (needs round (0,1)'s
            #       normalize) - woven into round (1,1)
            fl_a = []
            for mi in range(DK // 128):
                for ss in range(2):
                    fl_a.append(
                        fl_quarter(
                            wo_sb[:, ts(mi, 128)],
                            oT_all[:, ts(ss, 512)],
                            PT.ap()[ts(mi, 128), ts(ss, 512)],
                        )
                    )
            fl_b = []
            for mi in range(DK // 128):
                for ss in range(2, 4):
                    fl_b.append(
                        fl_quarter(
                            wo_sb[0:DH, ts(mi, 128)],
                            oT_all[0:DH, ts(ss, 512)],
                            PT.ap()[ts(mi, 128), ts(ss, 512)],
                        )
                    )

            attention_round(0, 0)
            attention_round(1, 0)
            attention_round(0, 1, extras=fl_a[:10], extras_from=6)
            oc_last = attention_round(
                1, 1, extras=fl_a[10:] + fl_b[:8], extras_from=2
            )
            # remaining quarters + the head-1 contribution to the second
            # s-half, straight from the unnormalized o^T copy (no normalize
            # chain in the tail; host scales by 1/DN)
            tail_idx = 0
            for q in fl_b[8:]:
                q()
                tail_idx += 1
            for mi in range(DK // 128):
                for sl in range(2):
                    fl_quarter(
                        wo1_sb[:, ts(mi, 128)],
                        oc_last[0:DH, ts(sl, 512)],
                        PTB.ap()[ts(mi, 128), ts(sl, 512)],
                        use_act=(tail_idx % 2 == 1),
                        tag="ps" if tail_idx % 2 == 0 else "acc",
                    )()
                    tail_idx += 1

    nc.compile()
    return nc


def _get_nc():
    if "nc" not in _CACHE:
        rdt = F32 if os.environ.get("MHA_PREC", "f32r") == "f32" else F32R
        _CACHE["nc"] = _build_nc(rdt)
    return _CACHE["nc"]


def make_in_maps(Q, K, V, Wq, bq, Wk, bk, Wv, bv, Wo):
    in_maps = []
    for i in range(NCORES):
        c0 = i * CW
        h0, h1 = 2 * i, 2 * i + 1

        def blockdiag_t(W):
            out = np.zeros((CW, CW), np.float32)
            out[0:DH, 0:DH] = W[h0].T
            out[DH:CW, DH:CW] = W[h1].T
            return out

        in_maps.append(
            {
                "Qs": np.ascontiguousarray(Q[:, c0 : c0 + CW]),
                "Ks": np.ascontiguousarray(K[:, c0 : c0 + CW]),
                "Vs": np.ascontiguousarray(V[:, c0 : c0 + CW]),
                "Wtq": blockdiag_t(Wq),
                "Wtk": blockdiag_t(Wk),
                "Wtv": blockdiag_t(Wv),
                "Bq": np.concatenate([bq[h0], bq[h1]]).reshape(CW, 1).astype(np.float32),
                "Bk": np.concatenate([bk[h0], bk[h1]]).reshape(CW, 1).astype(np.float32),
                "Bv0": bv[h0].reshape(DH, 1).astype(np.float32),
                "Bv1": bv[h1].reshape(DH, 1).astype(np.float32),
                "WoT": np.ascontiguousarray(Wo[:, c0 : c0 + CW].T),
                "WoT1": np.ascontiguousarray(Wo[:, c0 + DH : c0 + CW].T),
            }
        )
    return in_maps


def kernel(Q, K, V, Wq, bq, Wk, bk, Wv, bv, Wo, bo, _spmd_kwargs=None):
    Q, K, V = (np.asarray(x, np.float32) for x in (Q, K, V))
    Wq, bq, Wk, bk, Wv, bv = (
        np.asarray(x, np.float32) for x in (Wq, bq, Wk, bk, Wv, bv)
    )
    Wo, bo = np.asarray(Wo, np.float32), np.asarray(bo, np.float32)

    nc = _get_nc()
    in_maps = make_in_maps(Q, K, V, Wq, bq, Wk, bk, Wv, bv, Wo)
    res = run_bass_kernel_spmd(
        nc, in_maps, core_ids=list(range(NCORES)), **(_spmd_kwargs or {})
    )

    # unshard: sum the row-parallel partials, add bo.  Each core's last-round
    # head contribution (PTB, columns S/2..S) comes back unnormalized with
    # its softmax denominator row DN - apply the 1/DN column scaling here.
    acc = np.zeros((DK, S), np.float64)
    for i in range(NCORES):
        r = res.results[i]
        acc += r["PT"]
        acc[:, S // 2 :] += r["PTB"].astype(np.float64) / r["DN"][0][None, :]
    out = (acc.T + bo).astype(np.float32)
    if _spmd_kwargs:
        return out, res
    return out



# revision 6
# speedup vs baseline: 1.2513x; 1.2513x over previous
"""Multi-head attention (16 heads, d_model=1024, S=2048) on 8 Trainium2 cores.

Sharding: tensor-parallel over heads — each core owns 2 heads (its slice of
Wq/Wk/Wv and the matching 128 columns of Q/K/V and of Wo).  Each core computes
its heads' attention and a row-parallel partial of the final linear; the host
sums the 8 partials and adds bo (the unshard step for row-parallel).

v2 design notes (vs the first working kernel):
  - Q/K/V slices are pre-transposed on the HOST into [feature, seq] layout,
    so the kernel loads T-space activations directly: no PE transposes at all
    (the v1 prologue spent ~50us at half PE clock on 80 transpose+drain
    chains, since transpose-mode doesn't count as HAM activity).
  - v is projected directly into [t, e] orientation (stationary = VT chunk,
    moving = Wv^T), landing in per-chunk stationaries [v0 | 1 | v1 | 1]
    (stride 130): head h's attn@V stationary is cols h*65 : h*65+65, and the
    ones column accumulates the softmax denominator for free (row 64 of the
    output for both heads).
  - bk is dropped entirely: softmax over t is invariant to the q.bk term
    (constant in t).  bv is folded at the v drain; bq stays (varies over t).
  - No PTB side-channel: the last round normalizes on device like the rest;
    all 32 final-linear quarters are K=128.  Output PT is written in fp16
    (host accumulates in float64), halving output DMA.
  - PE warmup matmuls on a zeroed tile + a 1-element exp are issued at t=0 so
    the HAM clock is at 2.4 GHz and the ACT exp table is resident by the time
    the input DMA completes.

Softmax is computed without max-subtraction: scores are bounded (|s|*0.125 <
~10), far from fp32 exp overflow, and softmax is shift-invariant.  Matmuls
use float32r (full-rate fp32 streaming, TF32-grade rounding).  Set
MHA_PREC=f32 for exact-fp32 matmuls (4x slower on the PE).
"""

import os
import sys

for _p in ("/opt/trn_rl_repo",):
    if _p not in sys.path:
        sys.path.insert(0, _p)

from contextlib import ExitStack

import numpy as np

import concourse.bass as bass
import concourse.tile as tile
from concourse import bacc, mybir
from concourse.bass import ts
from concourse.bass_utils import run_bass_kernel_spmd

S = 2048          # sequence length
DK = 1024         # d_model
H = 16            # heads
DH = 64           # head dim
NCORES = 8
CW = 128          # per-core feature slice width (2 heads x 64)
NCH = S // 128    # 16 chunks of 128 along the sequence
VSTRIDE = 2 * (DH + 1)  # 130: [v0 64 | ones | v1 64 | ones] per chunk

F32 = mybir.dt.float32
F16 = mybir.dt.float16
F32R = mybir.dt.float32r
EXP = mybir.ActivationFunctionType.Exp

N_WARMUP_MM = 20  # PE warmup matmuls at t=0 (~4.3us @2.4GHz)

_CACHE = {}


def _build_nc(rdt, out_dt):
    nc = bacc.Bacc(
        "TRN2", target_bir_lowering=False, debug=False, enable_asserts=False
    )

    def dma_cast(dst_ap, src_ap):
        # pure bitcast on the DRAM side when loading into f32r tiles
        nc.sync.dma_start(dst_ap, src_ap.bitcast(rdt) if rdt != F32 else src_ap)

    QTd = nc.dram_tensor("QTd", [CW, S], F32, kind="ExternalInput")
    KTd = nc.dram_tensor("KTd", [CW, S], F32, kind="ExternalInput")
    VTd = nc.dram_tensor("VTd", [CW, S], F32, kind="ExternalInput")
    Wtq = nc.dram_tensor("Wtq", [CW, CW], F32, kind="ExternalInput")
    Wtk = nc.dram_tensor("Wtk", [CW, CW], F32, kind="ExternalInput")
    Wtv = nc.dram_tensor("Wtv", [CW, CW], F32, kind="ExternalInput")
    Bq = nc.dram_tensor("Bq", [CW, 1], F32, kind="ExternalInput")
    Bvb = nc.dram_tensor("Bvb", [1, CW], F32, kind="ExternalInput")
    WoT = nc.dram_tensor("WoT", [CW, DK], F32, kind="ExternalInput")
    PT = nc.dram_tensor("PT", [DK, S], out_dt, kind="ExternalOutput")

    with tile.TileContext(nc) as tc:
        with ExitStack() as ctx:
            pers = ctx.enter_context(tc.tile_pool(name="pers", bufs=1))
            expool = ctx.enter_context(tc.tile_pool(name="expool", bufs=3))
            stage = ctx.enter_context(tc.tile_pool(name="stage", bufs=4))
            nrm = ctx.enter_context(tc.tile_pool(name="nrm", bufs=2))
            psmm = ctx.enter_context(tc.tile_pool(name="psmm", bufs=2, space="PSUM"))
            psacc = ctx.enter_context(tc.tile_pool(name="psacc", bufs=2, space="PSUM"))
            dscr = ctx.enter_context(tc.tile_pool(name="dscr", bufs=2, space="DRAM"))

            # ---- t=0: warmups.  A zeroed tile feeds PE warmup matmuls (HAM
            # to 2.4 GHz before the real stream); a 1-col exp preloads the
            # ACT table set (~2.7us) under the input DMA.
            warm = pers.tile([128, 512], rdt, tag="warm")
            nc.gpsimd.memset(warm[:].bitcast(F32), 0.0)
            wex = pers.tile([128, 1], F32, tag="wex")
            nc.scalar.activation(wex[:], warm[:, 0:1].bitcast(F32), EXP)

            # ---- weights / biases (small, issued before the big loads) ----
            wtq_sb = pers.tile([CW, CW], rdt, tag="wtq")
            wtk_sb = pers.tile([CW, CW], rdt, tag="wtk")
            wtv_sb = pers.tile([CW, CW], rdt, tag="wtv")
            dma_cast(wtq_sb[:], Wtq.ap())
            dma_cast(wtk_sb[:], Wtk.ap())
            dma_cast(wtv_sb[:], Wtv.ap())
            bq_sb = pers.tile([CW, 1], F32, tag="bq")
            nc.sync.dma_start(bq_sb[:], Bq.ap())
            bvb = pers.tile([128, CW], F32, tag="bvb")
            nc.sync.dma_start(bvb[:], Bvb.ap().to_broadcast((128, CW)))
            wo_sb = pers.tile([CW, DK], rdt, tag="wo")
            dma_cast(wo_sb[:], WoT.ap())

            # ---- activation loads, already T-space: X[d2, s] ----
            # K first (kp gates the first scores matmul), then Q, then V.
            QT = pers.tile([128, S], rdt, tag="QT")
            KT = pers.tile([128, S], rdt, tag="KT")
            VT = pers.tile([128, S], rdt, tag="VT")
            for xt, dram in ((KT, KTd), (QT, QTd), (VT, VTd)):
                src = dram.ap().bitcast(rdt) if rdt != F32 else dram.ap()
                for sl in range(4):  # 512-col pieces so consumers start early
                    nc.sync.dma_start(xt[:, ts(sl, 512)], src[:, ts(sl, 512)])

            # ---- PE warmup stream (no readers; ring-rotates psmm) ----
            for _ in range(N_WARMUP_MM):
                pw = psmm.tile([128, 512], F32, tag="ps")
                nc.tensor.matmul(pw[:], warm[:, 0:128], warm[:])

            # ---- projections (T-space): xT = blockdiag(W.T) @ XT + b ----
            # k lands in per-head tiles with the other head's rows zeroed so
            # the scores matmuls can use K=128 stationaries (full PE-array
            # activity keeps the HAM clock at 2.4 GHz).
            qTs = pers.tile([128, S], rdt, tag="qTs")
            kp0 = pers.tile([128, S], rdt, tag="kp0")
            kp1 = pers.tile([128, S], rdt, tag="kp1")
            kp = [kp0, kp1]
            nc.gpsimd.memset(kp0[:].bitcast(F32), 0.0)
            nc.gpsimd.memset(kp1[:].bitcast(F32), 0.0)
            for sl in range(S // 512):
                pp = psmm.tile([128, 512], F32, tag="ps")
                nc.tensor.matmul(pp[:], wtk_sb[:], KT[:, ts(sl, 512)])
                # split the drains across both engines
                nc.vector.tensor_copy(kp0[0:DH, ts(sl, 512)], pp[0:DH, :])
                nc.scalar.copy(kp1[DH:128, ts(sl, 512)], pp[DH:128, :])
            for sl in range(S // 512):
                pp = psmm.tile([128, 512], F32, tag="ps")
                nc.tensor.matmul(pp[:], wtq_sb[:], QT[:, ts(sl, 512)])
                nc.vector.tensor_scalar_add(qTs[:, ts(sl, 512)], pp[:], bq_sb[:])

            # ---- v directly in [t, e]: per-chunk stationaries with a shared
            # ones column: [v0 64 | 1 | v1 64] (stride 129; padded for the
            # rearrange window of the last chunk).
            vst = pers.tile([128, NCH * VSTRIDE], rdt, tag="vst")
            ones_cols = vst[:].rearrange(
                "p (j b c) -> p j b c", j=NCH, b=2, c=DH + 1
            )[:, :, :, DH : DH + 1]
            nc.gpsimd.memset(ones_cols.bitcast(F32), 1.0)
            for j in range(NCH):
                pv = psmm.tile([128, 128], F32, tag="ps")
                nc.tensor.matmul(pv[:], VT[:, ts(j, 128)], wtv_sb[:])
                base = j * VSTRIDE
                dst = vst[:, base : base + VSTRIDE].rearrange(
                    "p (b c) -> p b c", b=2, c=DH + 1
                )[:, :, 0:DH]
                src = pv[:].rearrange("p (b c) -> p b c", b=2, c=DH)
                bias = bvb[:].rearrange("p (b c) -> p b c", b=2, c=DH)
                nc.vector.tensor_add(dst, src, bias)

            def vstat(h, j):
                base = j * VSTRIDE + h * (DH + 1)
                return vst[:, base : base + DH + 1]

            # ---- attention, per head, per s-half ----
            oT_all = pers.tile([128, S], rdt, tag="oT")

            def attention_round(h, sh, extras=(), extras_from=6, last=False):
                """One (head, s-half) round, software-pipelined: MM3 for
                chunk j-1 is emitted after MM2 of chunk j so the in-order PE
                queue never stalls on the current chunk's exp.  `extras` are
                final-linear quarter closures woven one per chunk (from chunk
                `extras_from`) into the ACT-gated PE slack."""
                hs = h * DH
                s0 = sh * 1024
                acc = psacc.tile([128, 1024], F32, tag="acc")
                exs = [None] * NCH
                extras = list(extras)

                def mm2(j):
                    sc = psmm.tile([128, 1024], F32, tag="ps")
                    for n in range(2):
                        nc.tensor.matmul(
                            sc[:, ts(n, 512)],
                            kp[h][:, ts(j, 128)],
                            qTs[:, s0 + n * 512 : s0 + (n + 1) * 512],
                        )
                    ex = expool.tile([128, 1024], rdt, tag="ex")
                    nc.scalar.activation(ex[:], sc[:], EXP, scale=0.125)
                    exs[j] = ex

                def mm3(j):
                    for n in range(2):
                        nc.tensor.matmul(
                            acc[0 : DH + 1, ts(n, 512)],
                            vstat(h, j),
                            exs[j][:, ts(n, 512)],
                            start=(j == 0),
                            stop=(j == NCH - 1),
                        )

                mm2(0)
                for j in range(1, NCH):
                    mm2(j)
                    mm3(j - 1)
                    if j >= extras_from and extras:
                        extras.pop(0)()
                mm3(NCH - 1)
                while extras:
                    extras.pop(0)()

                # acc rows (both heads): o in 0:64, denom in row 64
                den_row = DH
                o_rows = (0, DH)
                oc = nrm.tile([DH + 1, 1024], F32, tag="oc")
                nc.vector.tensor_copy(oc[:], acc[0 : DH + 1, :])
                # reciprocal of the denominator with all 64 lanes busy: bounce
                # the [1,1024] row to [64,16] and back via DRAM, then
                # partition-broadcast.  All off the PE/ACT queues.
                dnd = dscr.tile([1, 1024], F32, tag="dnd")
                nc.sync.dma_start(dnd[:], oc[den_row : den_row + 1, :])
                d16 = nrm.tile([DH, 16], F32, tag="d16")
                nc.sync.dma_start(
                    d16[:], dnd[0:1, :].rearrange("a (p f) -> (a p) f", p=DH)
                )
                r16 = nrm.tile([DH, 16], F32, tag="r16")
                nc.vector.reciprocal(r16[:], d16[:])
                rnd = dscr.tile([1, 1024], F32, tag="rnd")
                nc.sync.dma_start(
                    rnd[0:1, :].rearrange("a (p f) -> (a p) f", p=DH), r16[:]
                )
                rb = nrm.tile([DH, 1024], F32, tag="rb")
                nc.sync.dma_start(rb[:], rnd[0:1, :].to_broadcast((DH, 1024)))
                nc.vector.tensor_mul(
                    oT_all[hs : hs + DH, s0 : s0 + 1024],
                    oc[o_rows[0] : o_rows[1], :],
                    rb[:],
                )

            def fl_quarter(mi, ss, use_act=False, tag="acc"):
                """One [128, 512] quarter of the final-linear partial:
                matmul -> stage copy (fp16 convert) -> DMA out."""

                def emit():
                    p = (psacc if tag == "acc" else psmm).tile(
                        [128, 512], F32, tag=tag
                    )
                    nc.tensor.matmul(
                        p[:], wo_sb[:, ts(mi, 128)], oT_all[:, ts(ss, 512)]
                    )
                    st = stage.tile([128, 512], out_dt, tag="st")
                    if use_act:
                        nc.scalar.copy(st[:], p[:])
                    else:
                        nc.vector.tensor_copy(st[:], p[:])
                    nc.sync.dma_start(PT.ap()[ts(mi, 128), ts(ss, 512)], st[:])

                return emit

            # final-linear quarters for the first s-half (ss 0,1): ready
            # after round 2's normalize; woven into rounds 3 and 4.
            fl_a = [
                fl_quarter(mi, ss) for mi in range(DK // 128) for ss in range(2)
            ]

            attention_round(0, 0)
            attention_round(1, 0)
            attention_round(0, 1, extras=fl_a[:10], extras_from=6)
            attention_round(1, 1, extras=fl_a[10:], extras_from=2, last=True)

            # tail: quarters for ss 2,3 (need both heads' last normalizes)
            ti = 0
            for mi in range(DK // 128):
                for ss in range(2, 4):
                    fl_quarter(
                        mi,
                        ss,
                        use_act=(ti % 2 == 1),
                        tag="ps" if ti % 2 == 0 else "acc",
                    )()
                    ti += 1

    nc.compile()
    return nc


def _get_nc():
    if "nc" not in _CACHE:
        rdt = F32 if os.environ.get("MHA_PREC", "f32r") == "f32" else F32R
        out_dt = F32 if os.environ.get("MHA_OUT", "f16") == "f32" else F16
        _CACHE["nc"] = _build_nc(rdt, out_dt)
    return _CACHE["nc"]


def make_in_maps(Q, K, V, Wq, bq, Wk, bk, Wv, bv, Wo):
    in_maps = []
    for i in range(NCORES):
        c0 = i * CW
        h0, h1 = 2 * i, 2 * i + 1

        def blockdiag_t(W):
            out = np.zeros((CW, CW), np.float32)
            out[0:DH, 0:DH] = W[h0].T
            out[DH:CW, DH:CW] = W[h1].T
            return out

        in_maps.append(
            {
                "QTd": np.ascontiguousarray(Q[:, c0 : c0 + CW].T),
                "KTd": np.ascontiguousarray(K[:, c0 : c0 + CW].T),
                "VTd": np.ascontiguousarray(V[:, c0 : c0 + CW].T),
                "Wtq": blockdiag_t(Wq),
                "Wtk": blockdiag_t(Wk),
                "Wtv": blockdiag_t(Wv),
                "Bq": np.concatenate([bq[h0], bq[h1]]).reshape(CW, 1).astype(np.float32),
                "Bvb": np.concatenate([bv[h0], bv[h1]]).reshape(1, CW).astype(np.float32),
                "WoT": np.ascontiguousarray(Wo[:, c0 : c0 + CW].T),
            }
        )
    return in_maps


def kernel(Q, K, V, Wq, bq, Wk, bk, Wv, bv, Wo, bo, _spmd_kwargs=None):
    Q, K, V = (np.asarray(x, np.float32) for x in (Q, K, V))
    Wq, bq, Wk, bk, Wv, bv = (
        np.asarray(x, np.float32) for x in (Wq, bq, Wk, bk, Wv, bv)
    )
    Wo, bo = np.asarray(Wo, np.float32), np.asarray(bo, np.float32)

    nc = _get_nc()
    in_maps = make_in_maps(Q, K, V, Wq, bq, Wk, bk, Wv, bv, Wo)
    res = run_bass_kernel_spmd(
        nc, in_maps, core_ids=list(range(NCORES)), **(_spmd_kwargs or {})
    )

    # unshard: sum the row-parallel partials, add bo
    acc = np.zeros((DK, S), np.float64)
    for i in range(NCORES):
        acc += res.results[i]["PT"].astype(np.float64)
    out = (acc.T + bo).astype(np.float32)
    if _spmd_kwargs:
        return out, res
    return out


# revision 9
# speedup vs baseline: 1.2546x; 1.0026x over previous
"""Multi-head attention (16 heads, d_model=1024, S=2048) on 8 Trainium2 cores.

Sharding: tensor-parallel over heads — each core owns 2 heads (its slice of
Wq/Wk/Wv and the matching 128 columns of Q/K/V and of Wo).  Each core computes
its heads' attention and a row-parallel partial of the final linear; the host
sums the 8 partials and adds bo (the unshard step for row-parallel).

v2 design notes (vs the first working kernel):
  - Q/K/V slices are pre-transposed on the HOST into [feature, seq] layout,
    so the kernel loads T-space activations directly: no PE transposes at all
    (the v1 prologue spent ~50us at half PE clock on 80 transpose+drain
    chains, since transpose-mode doesn't count as HAM activity).
  - v is projected directly into [t, e] orientation (stationary = VT chunk,
    moving = Wv^T), landing in per-chunk stationaries [v0 | 1 | v1 | 1]
    (stride 130): head h's attn@V stationary is cols h*65 : h*65+65, and the
    ones column accumulates the softmax denominator for free (row 64 of the
    output for both heads).
  - bk is dropped entirely: softmax over t is invariant to the q.bk term
    (constant in t).  bv is folded at the v drain; bq stays (varies over t).
  - No PTB side-channel: the last round normalizes on device like the rest;
    all 32 final-linear quarters are K=128.  Output PT is written in fp16
    (host accumulates in float64), halving output DMA.
  - PE warmup matmuls on a zeroed tile + a 1-element exp are issued at t=0 so
    the HAM clock is at 2.4 GHz and the ACT exp table is resident by the time
    the input DMA completes.

Softmax is computed without max-subtraction: scores are bounded (|s|*0.125 <
~10), far from fp32 exp overflow, and softmax is shift-invariant.  Matmuls
use float32r (full-rate fp32 streaming, TF32-grade rounding).  Set
MHA_PREC=f32 for exact-fp32 matmuls (4x slower on the PE).
"""

import os
import sys

for _p in ("/opt/trn_rl_repo",):
    if _p not in sys.path:
        sys.path.insert(0, _p)

from contextlib import ExitStack

import numpy as np

import concourse.bass as bass
import concourse.tile as tile
from concourse import bacc, mybir
from concourse.bass import ts
from concourse.bass_utils import run_bass_kernel_spmd

S = 2048          # sequence length
DK = 1024         # d_model
H = 16            # heads
DH = 64           # head dim
NCORES = 8
CW = 128          # per-core feature slice width (2 heads x 64)
NCH = S // 128    # 16 chunks of 128 along the sequence
VSTRIDE = 2 * (DH + 1)  # 130: [v0 64 | ones | v1 64 | ones] per chunk

F32 = mybir.dt.float32
F16 = mybir.dt.float16
U16 = mybir.dt.uint16
F32R = mybir.dt.float32r
EXP = mybir.ActivationFunctionType.Exp

# fp16-bits Schraudolph exp for the DVE-offloaded chunks:
#   bits16(exp(0.125*s)) ~= s * SCH_A + SCH_B   (max rel err ~2.9%)
SCH_A = float(np.float32(0.125 * np.log2(np.e) * 2**10))
SCH_B = float(np.float32((15 - 0.0436775) * 2**10))
# chunks per round whose exp runs on the DVE instead of ACT
OFFLOAD = {0: (3, 7, 11, 15), 1: (3, 7, 11, 15), 2: (4, 9, 14), 3: (4, 9, 14)}

N_WARMUP_MM = 12  # PE warmup matmuls at t=0 (~2.6us @2.4GHz)

_CACHE = {}


def _build_nc(rdt, out_dt):
    nc = bacc.Bacc(
        "TRN2", target_bir_lowering=False, debug=False, enable_asserts=False
    )

    def dma_cast(dst_ap, src_ap):
        # pure bitcast on the DRAM side when loading into f32r tiles
        nc.sync.dma_start(dst_ap, src_ap.bitcast(rdt) if rdt != F32 else src_ap)

    QTd = nc.dram_tensor("QTd", [CW, S], F32, kind="ExternalInput")
    KTd = nc.dram_tensor("KTd", [CW, S], F32, kind="ExternalInput")
    VTd = nc.dram_tensor("VTd", [CW, S], F32, kind="ExternalInput")
    # packed weights: [wtq 128 | wtk 128 | wtv 128 | bq 1 | wo 1024]
    WP = nc.dram_tensor("WP", [CW, 3 * CW + 1 + DK], F32, kind="ExternalInput")
    Bvb = nc.dram_tensor("Bvb", [1, CW], F32, kind="ExternalInput")
    PT = nc.dram_tensor("PT", [DK, S], out_dt, kind="ExternalOutput")

    with tile.TileContext(nc) as tc:
        with ExitStack() as ctx:
            pers = ctx.enter_context(tc.tile_pool(name="pers", bufs=1))
            expool = ctx.enter_context(tc.tile_pool(name="expool", bufs=3))
            stage = ctx.enter_context(tc.tile_pool(name="stage", bufs=4))
            nrm = ctx.enter_context(tc.tile_pool(name="nrm", bufs=2))
            psmm = ctx.enter_context(tc.tile_pool(name="psmm", bufs=2, space="PSUM"))
            psacc = ctx.enter_context(tc.tile_pool(name="psacc", bufs=2, space="PSUM"))
            dscr = ctx.enter_context(tc.tile_pool(name="dscr", bufs=2, space="DRAM"))

            # ---- t=0: warmups.  A zeroed tile feeds PE warmup matmuls (HAM
            # to 2.4 GHz before the real stream); a 1-col exp preloads the
            # ACT table set (~2.7us) under the input DMA.
            warm = pers.tile([128, 512], rdt, tag="warm")
            nc.gpsimd.memset(warm[:].bitcast(F32), 0.0)
            wex = pers.tile([128, 1], F32, tag="wex")
            nc.scalar.activation(wex[:], warm[:, 0:1].bitcast(F32), EXP)

            # ---- weights: one packed DMA + the bvb broadcast ----
            wp = pers.tile([CW, 3 * CW + 1 + DK], rdt, tag="wp")
            dma_cast(wp[:], WP.ap())
            wtq_sb = wp[:, 0:CW]
            wtk_sb = wp[:, CW : 2 * CW]
            wtv_sb = wp[:, 2 * CW : 3 * CW]
            bq_sb = wp[:, 3 * CW : 3 * CW + 1].bitcast(F32)
            wo_sb = wp[:, 3 * CW + 1 :]
            bvb = pers.tile([128, CW], F32, tag="bvb")
            nc.scalar.dma_start(bvb[:], Bvb.ap().to_broadcast((128, CW)))

            # ---- activation loads, already T-space: X[d2, s] ----
            # K first (kp gates the first scores matmul), then Q, then V.
            QT = pers.tile([128, S], rdt, tag="QT")
            KT = pers.tile([128, S], rdt, tag="KT")
            VT = pers.tile([128, S], rdt, tag="VT")
            for xt, dram in ((KT, KTd), (QT, QTd), (VT, VTd)):
                src = dram.ap().bitcast(rdt) if rdt != F32 else dram.ap()
                # halves; triggers split across the two HW-DGE engines
                nc.sync.dma_start(xt[:, ts(0, 1024)], src[:, ts(0, 1024)])
                nc.scalar.dma_start(xt[:, ts(1, 1024)], src[:, ts(1, 1024)])

            # ---- PE warmup stream (no readers; ring-rotates psmm) ----
            for _ in range(N_WARMUP_MM):
                pw = psmm.tile([128, 512], F32, tag="ps")
                nc.tensor.matmul(pw[:], warm[:, 0:128], warm[:])

            # ---- projections (T-space): xT = blockdiag(W.T) @ XT + b ----
            # k lands in per-head tiles with the other head's rows zeroed so
            # the scores matmuls can use K=128 stationaries (full PE-array
            # activity keeps the HAM clock at 2.4 GHz).
            qTs = pers.tile([128, S], rdt, tag="qTs")
            kp0 = pers.tile([128, S], rdt, tag="kp0")
            kp1 = pers.tile([128, S], rdt, tag="kp1")
            kp = [kp0, kp1]
            nc.gpsimd.memset(kp0[:].bitcast(F32), 0.0)
            nc.gpsimd.memset(kp1[:].bitcast(F32), 0.0)
            for sl in range(S // 512):
                pp = psmm.tile([128, 512], F32, tag="ps")
                nc.tensor.matmul(pp[:], wtk_sb[:], KT[:, ts(sl, 512)])
                # split the drains across both engines
                nc.vector.tensor_copy(kp0[0:DH, ts(sl, 512)], pp[0:DH, :])
                nc.scalar.copy(kp1[DH:128, ts(sl, 512)], pp[DH:128, :])
            for sl in range(S // 512):
                pp = psmm.tile([128, 512], F32, tag="ps")
                nc.tensor.matmul(pp[:], wtq_sb[:], QT[:, ts(sl, 512)])
                nc.vector.tensor_scalar_add(qTs[:, ts(sl, 512)], pp[:], bq_sb[:])

            # ---- v directly in [t, e]: per-chunk stationaries with a shared
            # ones column: [v0 64 | 1 | v1 64] (stride 129; padded for the
            # rearrange window of the last chunk).
            vst = pers.tile([128, NCH * VSTRIDE], F16, tag="vst")
            ones_cols = vst[:].rearrange(
                "p (j b c) -> p j b c", j=NCH, b=2, c=DH + 1
            )[:, :, :, DH : DH + 1]
            nc.gpsimd.memset(ones_cols, 1.0)
            for j in range(NCH):
                pv = psmm.tile([128, 128], F32, tag="ps")
                nc.tensor.matmul(pv[:], VT[:, ts(j, 128)], wtv_sb[:])
                base = j * VSTRIDE
                dst = vst[:, base : base + VSTRIDE].rearrange(
                    "p (b c) -> p b c", b=2, c=DH + 1
                )[:, :, 0:DH]
                src = pv[:].rearrange("p (b c) -> p b c", b=2, c=DH)
                bias = bvb[:].rearrange("p (b c) -> p b c", b=2, c=DH)
                nc.vector.tensor_add(dst, src, bias)

            def vstat(h, j):
                base = j * VSTRIDE + h * (DH + 1)
                return vst[:, base : base + DH + 1]

            # ---- attention, per head, per s-half ----
            oT_all = pers.tile([128, S], rdt, tag="oT")

            def attention_round(h, sh, extras=(), extras_from=6, last=False):
                dge = nc.scalar if last else nc.sync
                """One (head, s-half) round, software-pipelined: MM3 for
                chunk j-1 is emitted after MM2 of chunk j so the in-order PE
                queue never stalls on the current chunk's exp.  `extras` are
                final-linear quarter closures woven one per chunk (from chunk
                `extras_from`) into the ACT-gated PE slack."""
                hs = h * DH
                s0 = sh * 1024
                acc = psacc.tile([128, 1024], F32, tag="acc")
                exs = [None] * NCH
                extras = list(extras)

                offload = OFFLOAD[2 * sh + h]

                def mm2(j):
                    sc = psmm.tile([128, 1024], F32, tag="ps")
                    for n in range(2):
                        nc.tensor.matmul(
                            sc[:, ts(n, 512)],
                            kp[h][:, ts(j, 128)],
                            qTs[:, s0 + n * 512 : s0 + (n + 1) * 512],
                        )
                    ex = expool.tile([128, 1024], F16, tag="ex")
                    if j in offload:
                        # Schraudolph: fp16 exp bits via one DVE mult+add
                        nc.vector.tensor_scalar(
                            ex[:].bitcast(U16), sc[:], SCH_A, SCH_B,
                            op0=mybir.AluOpType.mult, op1=mybir.AluOpType.add,
                        )
                    else:
                        nc.scalar.activation(ex[:], sc[:], EXP, scale=0.125)
                    exs[j] = ex

                def mm3(j):
                    for n in range(2):
                        nc.tensor.matmul(
                            acc[0 : DH + 1, ts(n, 512)],
                            vstat(h, j),
                            exs[j][:, ts(n, 512)],
                            start=(j == 0),
                            stop=(j == NCH - 1),
                        )

                mm2(0)
                for j in range(1, NCH):
                    mm2(j)
                    mm3(j - 1)
                    if j >= extras_from and extras:
                        extras.pop(0)()
                mm3(NCH - 1)
                while extras:
                    extras.pop(0)()

                # acc rows (both heads): o in 0:64, denom in row 64
                den_row = DH
                o_rows = (0, DH)
                oc = nrm.tile([DH + 1, 1024], F32, tag="oc")
                nc.vector.tensor_copy(oc[:], acc[0 : DH + 1, :])
                # reciprocal of the denominator with all 64 lanes busy: bounce
                # the [1,1024] row to [64,16] and back via DRAM, then
                # partition-broadcast.  All off the PE/ACT queues.
                dnd = dscr.tile([1, 1024], F32, tag="dnd")
                dge.dma_start(dnd[:], oc[den_row : den_row + 1, :])
                d16 = nrm.tile([DH, 16], F32, tag="d16")
                dge.dma_start(
                    d16[:], dnd[0:1, :].rearrange("a (p f) -> (a p) f", p=DH)
                )
                r16 = nrm.tile([DH, 16], F32, tag="r16")
                nc.vector.reciprocal(r16[:], d16[:])
                rnd = dscr.tile([1, 1024], F32, tag="rnd")
                dge.dma_start(
                    rnd[0:1, :].rearrange("a (p f) -> (a p) f", p=DH), r16[:]
                )
                rb = nrm.tile([DH, 1024], F32, tag="rb")
                dge.dma_start(rb[:], rnd[0:1, :].to_broadcast((DH, 1024)))
                nc.vector.tensor_mul(
                    oT_all[hs : hs + DH, s0 : s0 + 1024],
                    oc[o_rows[0] : o_rows[1], :],
                    rb[:],
                )

            def fl_quarter(mi, ss, use_act=False, tag="acc"):
                """One [128, 512] quarter of the final-linear partial:
                matmul -> stage copy (fp16 convert) -> DMA out."""

                def emit():
                    p = (psacc if tag == "acc" else psmm).tile(
                        [128, 512], F32, tag=tag
                    )
                    nc.tensor.matmul(
                        p[:], wo_sb[:, ts(mi, 128)], oT_all[:, ts(ss, 512)]
                    )
                    st = stage.tile([128, 512], out_dt, tag="st")
                    if use_act:
                        nc.scalar.copy(st[:], p[:])
                    else:
                        nc.vector.tensor_copy(st[:], p[:])
                    nc.sync.dma_start(PT.ap()[ts(mi, 128), ts(ss, 512)], st[:])

                return emit

            # final-linear quarters for the first s-half (ss 0,1): ready
            # after round 2's normalize; woven into rounds 3 and 4.
            fl_a = [
                fl_quarter(mi, ss) for mi in range(DK // 128) for ss in range(2)
            ]

            attention_round(0, 0)
            attention_round(1, 0)
            attention_round(0, 1, extras=fl_a[:10], extras_from=6)
            attention_round(1, 1, extras=fl_a[10:], extras_from=2, last=True)

            # tail: PE dummies keep the HAM clock warm while the last
            # normalize chain flows, then the ss 2,3 quarters run hot
            for _ in range(20):
                pw = psmm.tile([128, 512], F32, tag="ps")
                nc.tensor.matmul(pw[:], warm[:, 0:128], warm[:])
            ti = 0
            for mi in range(DK // 128):
                for ss in range(2, 4):
                    fl_quarter(
                        mi,
                        ss,
                        use_act=(ti % 2 == 1),
                        tag="ps" if ti % 2 == 0 else "acc",
                    )()
                    ti += 1

    nc.compile()
    return nc


def _get_nc():
    if "nc" not in _CACHE:
        rdt = F32 if os.environ.get("MHA_PREC", "f32r") == "f32" else F32R
        out_dt = F32 if os.environ.get("MHA_OUT", "f16") == "f32" else F16
        _CACHE["nc"] = _build_nc(rdt, out_dt)
    return _CACHE["nc"]


def make_in_maps(Q, K, V, Wq, bq, Wk, bk, Wv, bv, Wo):
    in_maps = []
    for i in range(NCORES):
        c0 = i * CW
        h0, h1 = 2 * i, 2 * i + 1

        def blockdiag_t(W):
            out = np.zeros((CW, CW), np.float32)
            out[0:DH, 0:DH] = W[h0].T
            out[DH:CW, DH:CW] = W[h1].T
            return out

        wpack = np.concatenate(
            [
                blockdiag_t(Wq),
                blockdiag_t(Wk),
                blockdiag_t(Wv),
                np.concatenate([bq[h0], bq[h1]]).reshape(CW, 1).astype(np.float32),
                np.ascontiguousarray(Wo[:, c0 : c0 + CW].T),
            ],
            axis=1,
        )
        in_maps.append(
            {
                "QTd": np.ascontiguousarray(Q[:, c0 : c0 + CW].T),
                "KTd": np.ascontiguousarray(K[:, c0 : c0 + CW].T),
                "VTd": np.ascontiguousarray(V[:, c0 : c0 + CW].T),
                "WP": np.ascontiguousarray(wpack),
                "Bvb": np.concatenate([bv[h0], bv[h1]]).reshape(1, CW).astype(np.float32),
            }
        )
    return in_maps


def kernel(Q, K, V, Wq, bq, Wk, bk, Wv, bv, Wo, bo, _spmd_kwargs=None):
    Q, K, V = (np.asarray(x, np.float32) for x in (Q, K, V))
    Wq, bq, Wk, bk, Wv, bv = (
        np.asarray(x, np.float32) for x in (Wq, bq, Wk, bk, Wv, bv)
    )
    Wo, bo = np.asarray(Wo, np.float32), np.asarray(bo, np.float32)

    nc = _get_nc()
    in_maps = make_in_maps(Q, K, V, Wq, bq, Wk, bk, Wv, bv, Wo)
    res = run_bass_kernel_spmd(
        nc, in_maps, core_ids=list(range(NCORES)), **(_spmd_kwargs or {})
    )

    # unshard: sum the row-parallel partials, add bo
    acc = np.zeros((DK, S), np.float64)
    for i in range(NCORES):
        acc += res.results[i]["PT"].astype(np.float64)
    out = (acc.T + bo).astype(np.float32)
    if _spmd_kwargs:
        return out, res
    return out


# revision 12
# speedup vs baseline: 1.3060x; 1.0410x over previous
"""Multi-head attention (16 heads, d_model=1024, S=2048) on 8 Trainium2 cores.

Sharding: tensor-parallel over heads — each core owns 2 heads (its slice of
Wq/Wk/Wv and the matching 128 columns of Q/K/V and of Wo).  Each core computes
its heads' attention and a row-parallel partial of the final linear; the host
sums the 8 partials and adds bo (the unshard step for row-parallel).

v4 design notes:
  - Everything the PE touches is BF16 (inputs pre-transposed AND pre-cast on
    the host): the bf16 moving-operand path streams faster than fp32/f32r and
    FWL applies to the weight loads.  PSUM stays fp32 (hardware requirement).
  - Q/K/V arrive in T-space [feature, seq]; no PE transposes anywhere.
  - v is projected directly into per-chunk [t, e] attn@V stationaries
    [v0 | 1 | v1 | 1] (ones columns accumulate softmax denominators); the 16
    small v-proj matmuls are WOVEN into round 1's PE stream (pre_extras).
  - bk is dropped (softmax over t is invariant to it); bv folds into the v
    drain; bq stays.
  - exp: ACT does most chunks (fp32 scores from PSUM -> bf16); 3-4 chunks per
    round are offloaded to the DVE as a one-op Schraudolph (bf16 exp bits =
    scores * SCH_A + SCH_B computed as uint16), +-2.9% on those weights only.
  - Final linear: 32 K=128 quarters for the s-range whose normalizes finish
    early (woven into rounds 3/4), and for the last round's half: head0's
    K=64 part is woven into round 4 (PT cols 1024:2048), head1's K=64 part is
    exported UNNORMALIZED as PTB with the fp32 denominator row DN — the host
    divides during the reduce.  No normalize chain in the tail at all.
"""

import os
import sys

for _p in ("/opt/trn_rl_repo",):
    if _p not in sys.path:
        sys.path.insert(0, _p)

from contextlib import ExitStack

import ml_dtypes
import numpy as np

import concourse.bass as bass
import concourse.tile as tile
from concourse import bacc, mybir
from concourse.bass import ts
from concourse.bass_utils import run_bass_kernel_spmd

S = 2048          # sequence length
DK = 1024         # d_model
H = 16            # heads
DH = 64           # head dim
NCORES = 8
CW = 128          # per-core feature slice width (2 heads x 64)
NCH = S // 128    # 16 chunks of 128 along the sequence
VSTRIDE = 2 * (DH + 1)  # 130: [v0 64 | ones | v1 64 | ones] per chunk

F32 = mybir.dt.float32
BF16 = mybir.dt.bfloat16
U16 = mybir.dt.uint16
EXP = mybir.ActivationFunctionType.Exp

# bf16-bits Schraudolph exp for the DVE-offloaded chunks:
#   bits16(exp(0.125*s)) ~= s * SCH_A + SCH_B  (max rel err ~2.9%)
SCH_A = float(np.float32(0.125 * np.log2(np.e) * 2**7))
SCH_B = float(np.float32((127 - 0.0436775) * 2**7))
# chunks per round whose exp runs on the DVE instead of ACT
OFFLOAD = {0: (3, 7, 11, 15), 1: (3, 7, 11, 15), 2: (4, 9, 14), 3: (4, 9, 14)}

N_WARMUP_MM = 8  # PE warmup matmuls at t=0

_CACHE = {}


def _build_nc():
    nc = bacc.Bacc(
        "TRN2", target_bir_lowering=False, debug=False, enable_asserts=False
    )

    QTd = nc.dram_tensor("QTd", [CW, S], BF16, kind="ExternalInput")
    KTd = nc.dram_tensor("KTd", [CW, S], BF16, kind="ExternalInput")
    VTd = nc.dram_tensor("VTd", [CW, S], BF16, kind="ExternalInput")
    # packed bf16 weights: [wtq 128 | wtk 128 | wtv 128 | wo 1024]
    WPH = nc.dram_tensor("WPH", [CW, 3 * CW + DK], BF16, kind="ExternalInput")
    Bq = nc.dram_tensor("Bq", [CW, 1], F32, kind="ExternalInput")
    Bvb = nc.dram_tensor("Bvb", [1, CW], F32, kind="ExternalInput")
    WO1 = nc.dram_tensor("WO1", [DH, DK], BF16, kind="ExternalInput")
    PT = nc.dram_tensor("PT", [DK, S], BF16, kind="ExternalOutput")
    # head-1 contribution to the last s-half, unnormalized; host divides by DN
    PTB = nc.dram_tensor("PTB", [DK, S // 2], BF16, kind="ExternalOutput")
    DN = nc.dram_tensor("DN", [1, S // 2], F32, kind="ExternalOutput")

    with tile.TileContext(nc) as tc:
        with ExitStack() as ctx:
            pers = ctx.enter_context(tc.tile_pool(name="pers", bufs=1))
            expool = ctx.enter_context(tc.tile_pool(name="expool", bufs=3))
            stage = ctx.enter_context(tc.tile_pool(name="stage", bufs=4))
            nrm = ctx.enter_context(tc.tile_pool(name="nrm", bufs=2))
            psmm = ctx.enter_context(tc.tile_pool(name="psmm", bufs=2, space="PSUM"))
            psacc = ctx.enter_context(tc.tile_pool(name="psacc", bufs=2, space="PSUM"))
            dscr = ctx.enter_context(tc.tile_pool(name="dscr", bufs=2, space="DRAM"))

            # ---- t=0: PE warmup fodder + ACT exp table preload ----
            warm = pers.tile([128, 512], BF16, tag="warm")
            nc.gpsimd.memset(warm[:], 0.0)
            wex = pers.tile([128, 1], F32, tag="wex")
            nc.scalar.activation(wex[:], warm[:, 0:1], EXP)

            # ---- weights (one packed DMA) + biases ----
            wp = pers.tile([CW, 3 * CW + DK], BF16, tag="wp")
            nc.sync.dma_start(wp[:], WPH.ap())
            wtq_sb = wp[:, 0:CW]
            wtk_sb = wp[:, CW : 2 * CW]
            wtv_sb = wp[:, 2 * CW : 3 * CW]
            wo_sb = wp[:, 3 * CW :]
            bq_sb = pers.tile([CW, 1], F32, tag="bq")
            nc.scalar.dma_start(bq_sb[:], Bq.ap())
            bvb = pers.tile([128, CW], F32, tag="bvb")
            nc.scalar.dma_start(bvb[:], Bvb.ap().to_broadcast((128, CW)))
            wo1_sb = pers.tile([DH, DK], BF16, tag="wo1")
            nc.scalar.dma_start(wo1_sb[:], WO1.ap())

            # ---- activation loads (T-space, bf16): K first, halves split
            # across the two HW-DGE trigger engines ----
            QT = pers.tile([128, S], BF16, tag="QT")
            KT = pers.tile([128, S], BF16, tag="KT")
            VT = pers.tile([128, S], BF16, tag="VT")
            for xt, dram in ((KT, KTd), (QT, QTd), (VT, VTd)):
                nc.sync.dma_start(xt[:, ts(0, 1024)], dram.ap()[:, ts(0, 1024)])
                nc.scalar.dma_start(xt[:, ts(1, 1024)], dram.ap()[:, ts(1, 1024)])

            # ---- PE warmup stream (no readers; rotates psmm ring) ----
            for _ in range(N_WARMUP_MM):
                pw = psmm.tile([128, 512], F32, tag="ps")
                nc.tensor.matmul(pw[:], warm[:, 0:128], warm[:])

            # ---- projections: xT = blockdiag(W.T) @ XT (+ b) ----
            # k lands per-head with the other head's rows zeroed (K=128
            # scores stationaries keep full PE-array activity).
            qTs = pers.tile([128, S], BF16, tag="qTs")
            kp0 = pers.tile([128, S], BF16, tag="kp0")
            kp1 = pers.tile([128, S], BF16, tag="kp1")
            kp = [kp0, kp1]
            nc.gpsimd.memset(kp0[:], 0.0)
            nc.gpsimd.memset(kp1[:], 0.0)
            for sl in range(S // 512):
                pp = psmm.tile([128, 512], F32, tag="ps")
                nc.tensor.matmul(pp[:], wtk_sb, KT[:, ts(sl, 512)])
                nc.vector.tensor_copy(kp0[0:DH, ts(sl, 512)], pp[0:DH, :])
                nc.scalar.copy(kp1[DH:128, ts(sl, 512)], pp[DH:128, :])
                # q right behind k so round 1 can start after slices 0,1
                pq = psmm.tile([128, 512], F32, tag="ps")
                nc.tensor.matmul(pq[:], wtq_sb, QT[:, ts(sl, 512)])
                nc.vector.tensor_scalar_add(qTs[:, ts(sl, 512)], pq[:], bq_sb[:])

            # ---- v in [t, e]: per-chunk stationaries with ones columns ----
            vst = pers.tile([128, NCH * VSTRIDE], BF16, tag="vst")
            ones_cols = vst[:].rearrange(
                "p (j b c) -> p j b c", j=NCH, b=2, c=DH + 1
            )[:, :, :, DH : DH + 1]
            nc.gpsimd.memset(ones_cols, 1.0)

            def vproj(j):
                def emit():
                    pv = psmm.tile([128, 128], F32, tag="ps")
                    nc.tensor.matmul(pv[:], VT[:, ts(j, 128)], wtv_sb)
                    base = j * VSTRIDE
                    dst = vst[:, base : base + VSTRIDE].rearrange(
                        "p (b c) -> p b c", b=2, c=DH + 1
                    )[:, :, 0:DH]
                    src = pv[:].rearrange("p (b c) -> p b c", b=2, c=DH)
                    bias = bvb[:].rearrange("p (b c) -> p b c", b=2, c=DH)
                    nc.vector.tensor_add(dst, src, bias)

                return emit

            vproj(0)()
            vproj(1)()

            def vstat(h, j):
                base = j * VSTRIDE + h * (DH + 1)
                return vst[:, base : base + DH + 1]

            # ---- attention, per head, per s-half ----
            oT_all = pers.tile([128, S], BF16, tag="oT")

            def attention_round(
                h, sh, extras=(), extras_from=6, pre_extras=(), last=False
            ):
                """One (head, s-half) round, software-pipelined: MM3 for
                chunk j-1 is emitted after MM2 of chunk j.  `pre_extras` fire
                between MM2(j) and MM3(j-1) (used to weave round 1's v-proj
                two chunks ahead of its consumer); `extras` fire after
                MM3(j-1) from chunk `extras_from` (final-linear quarters)."""
                hs = h * DH
                s0 = sh * 1024
                acc = psacc.tile([128, 1024], F32, tag="acc")
                exs = [None] * NCH
                extras = list(extras)
                pre_extras = list(pre_extras)
                offload = OFFLOAD[2 * sh + h]

                def mm2(j):
                    sc = psmm.tile([128, 1024], F32, tag="ps")
                    for n in range(2):
                        nc.tensor.matmul(
                            sc[:, ts(n, 512)],
                            kp[h][:, ts(j, 128)],
                            qTs[:, s0 + n * 512 : s0 + (n + 1) * 512],
                        )
                    ex = expool.tile([128, 1024], BF16, tag="ex")
                    if j in offload:
                        nc.vector.tensor_scalar(
                            ex[:].bitcast(U16), sc[:], SCH_A, SCH_B,
                            op0=mybir.AluOpType.mult, op1=mybir.AluOpType.add,
                        )
                    else:
                        nc.scalar.activation(ex[:], sc[:], EXP, scale=0.125)
                    exs[j] = ex

                def mm3(j):
                    for n in range(2):
                        nc.tensor.matmul(
                            acc[0 : DH + 1, ts(n, 512)],
                            vstat(h, j),
                            exs[j][:, ts(n, 512)],
                            start=(j == 0),
                            stop=(j == NCH - 1),
                        )

                mm2(0)
                for j in range(1, NCH):
                    mm2(j)
                    if pre_extras:
                        pre_extras.pop(0)()
                    mm3(j - 1)
                    if j >= extras_from and extras:
                        extras.pop(0)()
                mm3(NCH - 1)
                while extras:
                    extras.pop(0)()

                # acc rows (both heads): o in 0:64, denom in row 64
                if last:
                    # bf16 copy for the PTB quarters' moving operand + fp32
                    # denominator row for the host-side divide
                    ocb = nrm.tile([DH + 1, 1024], BF16, tag="ocb")
                    nc.vector.tensor_copy(ocb[:], acc[0 : DH + 1, :])
                    dn32 = nrm.tile([1, 1024], F32, tag="dn32")
                    nc.scalar.copy(dn32[0:1, :], acc[DH : DH + 1, :])
                    nc.scalar.dma_start(DN.ap(), dn32[0:1, :])
                    return ocb
                oc = nrm.tile([DH + 1, 1024], F32, tag="oc")
                nc.vector.tensor_copy(oc[:], acc[0 : DH + 1, :])
                # spread the denominator over 64 lanes for the reciprocal,
                # via DRAM (engines can't partition-scatter); broadcast back.
                dnd = dscr.tile([1, 1024], F32, tag="dnd")
                nc.sync.dma_start(dnd[:], oc[DH : DH + 1, :])
                d16 = nrm.tile([DH, 16], F32, tag="d16")
                nc.sync.dma_start(
                    d16[:], dnd[0:1, :].rearrange("a (p f) -> (a p) f", p=DH)
                )
                r16 = nrm.tile([DH, 16], F32, tag="r16")
                nc.vector.reciprocal(r16[:], d16[:])
                rnd = dscr.tile([1, 1024], F32, tag="rnd")
                nc.sync.dma_start(
                    rnd[0:1, :].rearrange("a (p f) -> (a p) f", p=DH), r16[:]
                )
                rb = nrm.tile([DH, 1024], F32, tag="rb")
                nc.sync.dma_start(rb[:], rnd[0:1, :].to_broadcast((DH, 1024)))
                nc.vector.tensor_mul(
                    oT_all[hs : hs + DH, s0 : s0 + 1024],
                    oc[0:DH, :],
                    rb[:],
                )
                return None

            def fl_quarter(lhsT, rhs, out_slice, use_act=False, tag="acc"):
                """Final-linear quarter: matmul -> bf16 stage copy -> DMA."""

                def emit():
                    p = (psacc if tag == "acc" else psmm).tile(
                        [128, 512], F32, tag=tag
                    )
                    nc.tensor.matmul(p[:], lhsT, rhs)
                    st = stage.tile([128, 512], BF16, tag="st")
                    if use_act:
                        nc.scalar.copy(st[:], p[:])
                    else:
                        nc.vector.tensor_copy(st[:], p[:])
                    nc.sync.dma_start(out_slice, st[:])

                return emit

            # quarters for s-half 0 (both heads, K=128): woven into rounds 3,4
            fl_a = [
                fl_quarter(
                    wo_sb[:, ts(mi, 128)],
                    oT_all[:, ts(ss, 512)],
                    PT.ap()[ts(mi, 128), ts(ss, 512)],
                )
                for mi in range(DK // 128)
                for ss in range(2)
            ]
            # head0's K=64 part of s-half 1 (needs round 3's normalize):
            # woven into round 4; completes PT cols 1024:2048
            fl_b = [
                fl_quarter(
                    wo_sb[0:DH, ts(mi, 128)],
                    oT_all[0:DH, ts(ss, 512)],
                    PT.ap()[ts(mi, 128), ts(ss, 512)],
                )
                for mi in range(DK // 128)
                for ss in range(2, 4)
            ]

            attention_round(0, 0, pre_extras=[vproj(j) for j in range(2, NCH)])
            attention_round(1, 0)
            attention_round(0, 1, extras=fl_a[:10], extras_from=6)
            ocb = attention_round(
                1, 1, extras=fl_a[10:] + fl_b, extras_from=2, last=True
            )

            # tail: head1's unnormalized K=64 quarters (host divides by DN)
            ti = 0
            for mi in range(DK // 128):
                for sl in range(2):
                    fl_quarter(
                        wo1_sb[:, ts(mi, 128)],
                        ocb[0:DH, ts(sl, 512)],
                        PTB.ap()[ts(mi, 128), ts(sl, 512)],
                        use_act=(ti % 2 == 1),
                        tag="ps" if ti % 2 == 0 else "acc",
                    )()
                    ti += 1

    nc.compile()
    return nc


def _get_nc():
    if "nc" not in _CACHE:
        _CACHE["nc"] = _build_nc()
    return _CACHE["nc"]


def make_in_maps(Q, K, V, Wq, bq, Wk, bk, Wv, bv, Wo):
    bf = ml_dtypes.bfloat16
    in_maps = []
    for i in range(NCORES):
        c0 = i * CW
        h0, h1 = 2 * i, 2 * i + 1

        def blockdiag_t(W):
            out = np.zeros((CW, CW), np.float32)
            out[0:DH, 0:DH] = W[h0].T
            out[DH:CW, DH:CW] = W[h1].T
            return out

        wpack = np.concatenate(
            [
                blockdiag_t(Wq),
                blockdiag_t(Wk),
                blockdiag_t(Wv),
                np.ascontiguousarray(Wo[:, c0 : c0 + CW].T),
            ],
            axis=1,
        )
        in_maps.append(
            {
                "QTd": np.ascontiguousarray(Q[:, c0 : c0 + CW].T).astype(bf),
                "KTd": np.ascontiguousarray(K[:, c0 : c0 + CW].T).astype(bf),
                "VTd": np.ascontiguousarray(V[:, c0 : c0 + CW].T).astype(bf),
                "WPH": wpack.astype(bf),
                "Bq": np.concatenate([bq[h0], bq[h1]]).reshape(CW, 1).astype(np.float32),
                "Bvb": np.concatenate([bv[h0], bv[h1]]).reshape(1, CW).astype(np.float32),
                "WO1": np.ascontiguousarray(Wo[:, c0 + DH : c0 + CW].T).astype(bf),
            }
        )
    return in_maps


def kernel(Q, K, V, Wq, bq, Wk, bk, Wv, bv, Wo, bo, _spmd_kwargs=None):
    Q, K, V = (np.asarray(x, np.float32) for x in (Q, K, V))
    Wq, bq, Wk, bk, Wv, bv = (
        np.asarray(x, np.float32) for x in (Wq, bq, Wk, bk, Wv, bv)
    )
    Wo, bo = np.asarray(Wo, np.float32), np.asarray(bo, np.float32)

    nc = _get_nc()
    in_maps = make_in_maps(Q, K, V, Wq, bq, Wk, bk, Wv, bv, Wo)
    res = run_bass_kernel_spmd(
        nc, in_maps, core_ids=list(range(NCORES)), **(_spmd_kwargs or {})
    )

    # unshard: sum the row-parallel partials, add bo.  PTB (head-1, second
    # s-half) comes back unnormalized; divide by its denominator row DN.
    acc = np.zeros((DK, S), np.float64)
    for i in range(NCORES):
        r = res.results[i]
        acc += r["PT"].astype(np.float64)
        acc[:, S // 2 :] += r["PTB"].astype(np.float64) / r["DN"][0].astype(np.float64)[None, :]
    out = (acc.T + bo).astype(np.float32)
    if _spmd_kwargs:
        return out, res
    return out


# revision 14
# speedup vs baseline: 1.3908x; 1.0649x over previous
"""Multi-head attention (16 heads, d_model=1024, S=2048) on 8 Trainium2 cores.

Sharding: tensor-parallel over heads — each core owns 2 heads (its slice of
Wq/Wk/Wv and the matching 128 columns of Q/K/V and of Wo).  Each core computes
its heads' attention and a row-parallel partial of the final linear; the host
sums the 8 partials and adds bo (the unshard step for row-parallel).

v4 design notes:
  - Everything the PE touches is BF16 (inputs pre-transposed AND pre-cast on
    the host): the bf16 moving-operand path streams faster than fp32/f32r and
    FWL applies to the weight loads.  PSUM stays fp32 (hardware requirement).
  - Q/K/V arrive in T-space [feature, seq]; no PE transposes anywhere.
  - v is projected directly into per-chunk [t, e] attn@V stationaries
    [v0 | 1 | v1 | 1] (ones columns accumulate softmax denominators); the 16
    small v-proj matmuls are WOVEN into round 1's PE stream (pre_extras).
  - bk is dropped (softmax over t is invariant to it); bv folds into the v
    drain; bq stays.
  - exp: ACT does most chunks (fp32 scores from PSUM -> bf16); 3-4 chunks per
    round are offloaded to the DVE as a one-op Schraudolph (bf16 exp bits =
    scores * SCH_A + SCH_B computed as uint16), +-2.9% on those weights only.
  - Final linear: 32 K=128 quarters for the s-range whose normalizes finish
    early (woven into rounds 3/4), and for the last round's half: head0's
    K=64 part is woven into round 4 (PT cols 1024:2048), head1's K=64 part is
    exported UNNORMALIZED as PTB with the fp32 denominator row DN — the host
    divides during the reduce.  No normalize chain in the tail at all.
"""

import os
import sys

for _p in ("/opt/trn_rl_repo",):
    if _p not in sys.path:
        sys.path.insert(0, _p)

from contextlib import ExitStack

import ml_dtypes
import numpy as np

import concourse.bass as bass
import concourse.tile as tile
from concourse import bacc, mybir
from concourse.bass import ts
from concourse.bass_utils import run_bass_kernel_spmd

S = 2048          # sequence length
DK = 1024         # d_model
H = 16            # heads
DH = 64           # head dim
NCORES = 8
CW = 128          # per-core feature slice width (2 heads x 64)
NCH = S // 128    # 16 chunks of 128 along the sequence
VSTRIDE = 2 * (DH + 1)  # 130: [v0 64 | ones | v1 64 | ones] per chunk

F32 = mybir.dt.float32
BF16 = mybir.dt.bfloat16
U16 = mybir.dt.uint16
EXP = mybir.ActivationFunctionType.Exp

# bf16-bits Schraudolph exp for the DVE-offloaded chunks:
#   bits16(exp(0.125*s)) ~= s * SCH_A + SCH_B  (max rel err ~2.9%)
SCH_A = float(np.float32(0.125 * np.log2(np.e) * 2**7))
SCH_B = float(np.float32((127 - 0.0436775) * 2**7))
# chunks per round whose exp runs on the DVE instead of ACT
OFFLOAD = {0: (3, 7, 11, 15), 1: (3, 7, 11, 15), 2: (4, 9, 14), 3: (4, 9, 14)}

N_WARMUP_MM = 6  # PE warmup matmuls at t=0

_CACHE = {}


def _build_nc():
    nc = bacc.Bacc(
        "TRN2", target_bir_lowering=False, debug=False, enable_asserts=False
    )

    QTd = nc.dram_tensor("QTd", [CW, S], BF16, kind="ExternalInput")
    KTd = nc.dram_tensor("KTd", [CW, S], BF16, kind="ExternalInput")
    VTd = nc.dram_tensor("VTd", [CW, S], BF16, kind="ExternalInput")
    # packed bf16 weights: [wtq 128 | wtk 128 | wtv 128 | wo 1024]
    WPH = nc.dram_tensor("WPH", [CW, 3 * CW + DK], BF16, kind="ExternalInput")
    Bq = nc.dram_tensor("Bq", [CW, 1], F32, kind="ExternalInput")
    Bvb = nc.dram_tensor("Bvb", [1, CW], F32, kind="ExternalInput")
    WO1 = nc.dram_tensor("WO1", [DH, DK], BF16, kind="ExternalInput")
    PT = nc.dram_tensor("PT", [DK, S], BF16, kind="ExternalOutput")
    # head-1 contribution to the last s-half, unnormalized; host divides by DN
    PTB = nc.dram_tensor("PTB", [DK, S // 2], BF16, kind="ExternalOutput")
    DN = nc.dram_tensor("DN", [1, S // 2], F32, kind="ExternalOutput")

    with tile.TileContext(nc) as tc:
        with ExitStack() as ctx:
            pers = ctx.enter_context(tc.tile_pool(name="pers", bufs=1))
            expool = ctx.enter_context(tc.tile_pool(name="expool", bufs=3))
            stage = ctx.enter_context(tc.tile_pool(name="stage", bufs=4))
            nrm = ctx.enter_context(tc.tile_pool(name="nrm", bufs=2))
            psmm = ctx.enter_context(tc.tile_pool(name="psmm", bufs=2, space="PSUM"))
            psacc = ctx.enter_context(tc.tile_pool(name="psacc", bufs=2, space="PSUM"))
            dscr = ctx.enter_context(tc.tile_pool(name="dscr", bufs=2, space="DRAM"))

            # ---- t=0: PE warmup fodder + ACT exp table preload ----
            warm = pers.tile([128, 512], BF16, tag="warm")
            nc.gpsimd.memset(warm[:], 0.0)
            wex = pers.tile([128, 1], F32, tag="wex")
            nc.scalar.activation(wex[:], warm[:, 0:1], EXP)

            # ---- loads: K halves first (they gate the scores stream),
            # then weights, then Q, V; the two HW-DGE engines' dynamic
            # queues run their transfers in parallel ----
            QT = pers.tile([128, S], BF16, tag="QT")
            KT = pers.tile([128, S], BF16, tag="KT")
            VT = pers.tile([128, S], BF16, tag="VT")
            wp = pers.tile([CW, 3 * CW + DK], BF16, tag="wp")
            bq_sb = pers.tile([CW, 1], F32, tag="bq")
            bvb = pers.tile([128, CW], F32, tag="bvb")
            wo1_sb = pers.tile([DH, DK], BF16, tag="wo1")

            nc.sync.dma_start(KT[:, ts(0, 1024)], KTd.ap()[:, ts(0, 1024)])
            nc.scalar.dma_start(KT[:, ts(1, 1024)], KTd.ap()[:, ts(1, 1024)])
            nc.sync.dma_start(wp[:], WPH.ap())
            nc.scalar.dma_start(bq_sb[:], Bq.ap())
            nc.scalar.dma_start(bvb[:], Bvb.ap().to_broadcast((128, CW)))
            nc.sync.dma_start(QT[:, ts(0, 1024)], QTd.ap()[:, ts(0, 1024)])
            nc.scalar.dma_start(QT[:, ts(1, 1024)], QTd.ap()[:, ts(1, 1024)])
            nc.sync.dma_start(VT[:, ts(0, 1024)], VTd.ap()[:, ts(0, 1024)])
            nc.scalar.dma_start(VT[:, ts(1, 1024)], VTd.ap()[:, ts(1, 1024)])
            nc.scalar.dma_start(wo1_sb[:], WO1.ap())
            wtq_sb = wp[:, 0:CW]
            wtk_sb = wp[:, CW : 2 * CW]
            wtv_sb = wp[:, 2 * CW : 3 * CW]
            wo_sb = wp[:, 3 * CW :]

            # ---- PE warmup stream (no readers; rotates psmm ring) ----
            for _ in range(N_WARMUP_MM):
                pw = psmm.tile([128, 512], F32, tag="ps")
                nc.tensor.matmul(pw[:], warm[:, 0:128], warm[:])

            # ---- projections: xT = blockdiag(W.T) @ XT (+ b) ----
            # k lands per-head with the other head's rows zeroed (K=128
            # scores stationaries keep full PE-array activity).
            qTs = pers.tile([128, S], BF16, tag="qTs")
            kp0 = pers.tile([128, S], BF16, tag="kp0")
            kp1 = pers.tile([128, S], BF16, tag="kp1")
            kp = [kp0, kp1]
            nc.vector.memset(kp0[:], 0.0)
            nc.vector.memset(kp1[:], 0.0)
            for sl in range(S // 512):
                pp = psmm.tile([128, 512], F32, tag="ps")
                nc.tensor.matmul(pp[:], wtk_sb, KT[:, ts(sl, 512)])
                nc.vector.tensor_copy(kp0[0:DH, ts(sl, 512)], pp[0:DH, :])
                nc.scalar.copy(kp1[DH:128, ts(sl, 512)], pp[DH:128, :])
                # q right behind k so round 1 can start after slices 0,1
                pq = psmm.tile([128, 512], F32, tag="ps")
                nc.tensor.matmul(pq[:], wtq_sb, QT[:, ts(sl, 512)])
                nc.vector.tensor_scalar_add(qTs[:, ts(sl, 512)], pq[:], bq_sb[:])

            # ---- v in [t, e]: per-chunk stationaries with ones columns ----
            vst = pers.tile([128, NCH * VSTRIDE], BF16, tag="vst")
            ones_cols = vst[:].rearrange(
                "p (j b c) -> p j b c", j=NCH, b=2, c=DH + 1
            )[:, :, :, DH : DH + 1]
            nc.gpsimd.memset(ones_cols, 1.0)

            def vproj(j):
                def emit():
                    pv = psmm.tile([128, 128], F32, tag="ps")
                    nc.tensor.matmul(pv[:], VT[:, ts(j, 128)], wtv_sb)
                    base = j * VSTRIDE
                    dst = vst[:, base : base + VSTRIDE].rearrange(
                        "p (b c) -> p b c", b=2, c=DH + 1
                    )[:, :, 0:DH]
                    src = pv[:].rearrange("p (b c) -> p b c", b=2, c=DH)
                    bias = bvb[:].rearrange("p (b c) -> p b c", b=2, c=DH)
                    nc.vector.tensor_add(dst, src, bias)

                return emit

            vproj(0)()
            vproj(1)()

            def vstat(h, j):
                base = j * VSTRIDE + h * (DH + 1)
                return vst[:, base : base + DH + 1]

            # ---- attention, per head, per s-half ----
            oT_all = pers.tile([128, S], BF16, tag="oT")

            def attention_round(
                h, sh, extras=(), extras_from=6, pre_extras=(), last=False
            ):
                """One (head, s-half) round, software-pipelined: MM3 for
                chunk j-1 is emitted after MM2 of chunk j.  `pre_extras` fire
                between MM2(j) and MM3(j-1) (used to weave round 1's v-proj
                two chunks ahead of its consumer); `extras` fire after
                MM3(j-1) from chunk `extras_from` (final-linear quarters)."""
                hs = h * DH
                s0 = sh * 1024
                acc = psacc.tile([128, 1024], F32, tag="acc")
                exs = [None] * NCH
                extras = list(extras)
                pre_extras = list(pre_extras)
                offload = OFFLOAD[2 * sh + h]

                def mm2(j):
                    sc = psmm.tile([128, 1024], F32, tag="ps")
                    for n in range(2):
                        nc.tensor.matmul(
                            sc[:, ts(n, 512)],
                            kp[h][:, ts(j, 128)],
                            qTs[:, s0 + n * 512 : s0 + (n + 1) * 512],
                        )
                    ex = expool.tile([128, 1024], BF16, tag="ex")
                    if j in offload:
                        nc.vector.tensor_scalar(
                            ex[:].bitcast(U16), sc[:], SCH_A, SCH_B,
                            op0=mybir.AluOpType.mult, op1=mybir.AluOpType.add,
                        )
                    else:
                        nc.scalar.activation(ex[:], sc[:], EXP, scale=0.125)
                    exs[j] = ex

                def mm3(j):
                    for n in range(2):
                        nc.tensor.matmul(
                            acc[0 : DH + 1, ts(n, 512)],
                            vstat(h, j),
                            exs[j][:, ts(n, 512)],
                            start=(j == 0),
                            stop=(j == NCH - 1),
                        )

                mm2(0)
                for j in range(1, NCH):
                    mm2(j)
                    if pre_extras:
                        pre_extras.pop(0)()
                    mm3(j - 1)
                    if j >= extras_from and extras:
                        extras.pop(0)()
                mm3(NCH - 1)
                while extras:
                    extras.pop(0)()

                # acc rows (both heads): o in 0:64, denom in row 64
                if last:
                    # bf16 copy for the PTB quarters' moving operand + fp32
                    # denominator row for the host-side divide
                    ocb = nrm.tile([DH + 1, 1024], BF16, tag="ocb")
                    nc.vector.tensor_copy(ocb[:], acc[0 : DH + 1, :])
                    dn32 = nrm.tile([1, 1024], F32, tag="dn32")
                    nc.scalar.copy(dn32[0:1, :], acc[DH : DH + 1, :])
                    nc.scalar.dma_start(DN.ap(), dn32[0:1, :])
                    return ocb
                oc = nrm.tile([DH + 1, 1024], F32, tag="oc")
                nc.vector.tensor_copy(oc[:], acc[0 : DH + 1, :])
                # spread the denominator over 64 lanes for the reciprocal,
                # via DRAM (engines can't partition-scatter); broadcast back.
                dnd = dscr.tile([1, 1024], F32, tag="dnd")
                nc.sync.dma_start(dnd[:], oc[DH : DH + 1, :])
                d16 = nrm.tile([DH, 16], F32, tag="d16")
                nc.sync.dma_start(
                    d16[:], dnd[0:1, :].rearrange("a (p f) -> (a p) f", p=DH)
                )
                r16 = nrm.tile([DH, 16], F32, tag="r16")
                nc.vector.reciprocal(r16[:], d16[:])
                rnd = dscr.tile([1, 1024], F32, tag="rnd")
                nc.sync.dma_start(
                    rnd[0:1, :].rearrange("a (p f) -> (a p) f", p=DH), r16[:]
                )
                rb = nrm.tile([DH, 1024], F32, tag="rb")
                nc.sync.dma_start(rb[:], rnd[0:1, :].to_broadcast((DH, 1024)))
                nc.vector.tensor_mul(
                    oT_all[hs : hs + DH, s0 : s0 + 1024],
                    oc[0:DH, :],
                    rb[:],
                )
                return None

            def fl_pair(lhsT, rhs0, rhs1, out_slice, tag="acc", dge=None):
                """Final-linear [128,1024] pair: two matmuls into one psum
                tile, bf16 stage casts split across DVE/ACT, one 2KB-line
                DMA."""

                def emit():
                    p = (psacc if tag == "acc" else psmm).tile(
                        [128, 1024], F32, tag=tag
                    )
                    nc.tensor.matmul(p[:, 0:512], lhsT, rhs0)
                    nc.tensor.matmul(p[:, 512:1024], lhsT, rhs1)
                    st = stage.tile([128, 1024], BF16, tag="st")
                    nc.vector.tensor_copy(st[:, 0:512], p[:, 0:512])
                    nc.scalar.copy(st[:, 512:1024], p[:, 512:1024])
                    (dge or nc.sync).dma_start(out_slice, st[:])

                return emit

            # pairs for s-half 0 (both heads, K=128): woven into round 3
            fl_a = [
                fl_pair(
                    wo_sb[:, ts(mi, 128)],
                    oT_all[:, ts(0, 512)],
                    oT_all[:, ts(1, 512)],
                    PT.ap()[ts(mi, 128), 0:1024],
                )
                for mi in range(DK // 128)
            ]
            # head0's K=64 part of s-half 1 (needs round 3's normalize):
            # woven into round 4; completes PT cols 1024:2048
            fl_b = [
                fl_pair(
                    wo_sb[0:DH, ts(mi, 128)],
                    oT_all[0:DH, ts(2, 512)],
                    oT_all[0:DH, ts(3, 512)],
                    PT.ap()[ts(mi, 128), 1024:2048],
                )
                for mi in range(DK // 128)
            ]

            attention_round(0, 0, pre_extras=[vproj(j) for j in range(2, NCH)])
            attention_round(1, 0)
            attention_round(0, 1, extras=fl_a, extras_from=6)
            ocb = attention_round(1, 1, extras=fl_b, extras_from=5, last=True)

            # a few dummies bridge the ocb copy so the PE stays warm
            for _ in range(4):
                pw = psmm.tile([128, 512], F32, tag="ps")
                nc.tensor.matmul(pw[:], warm[:, 0:128], warm[:])

            # tail: head1's unnormalized K=64 pairs (host divides by DN)
            for mi in range(DK // 128):
                fl_pair(
                    wo1_sb[:, ts(mi, 128)],
                    ocb[0:DH, ts(0, 512)],
                    ocb[0:DH, ts(1, 512)],
                    PTB.ap()[ts(mi, 128), 0:1024],
                    tag="ps" if mi % 2 == 0 else "acc",
                    dge=nc.sync if mi % 2 == 0 else nc.scalar,
                )()

    nc.compile()
    return nc


def _get_nc():
    if "nc" not in _CACHE:
        _CACHE["nc"] = _build_nc()
    return _CACHE["nc"]


def make_in_maps(Q, K, V, Wq, bq, Wk, bk, Wv, bv, Wo):
    bf = ml_dtypes.bfloat16
    in_maps = []
    for i in range(NCORES):
        c0 = i * CW
        h0, h1 = 2 * i, 2 * i + 1

        def blockdiag_t(W):
            out = np.zeros((CW, CW), np.float32)
            out[0:DH, 0:DH] = W[h0].T
            out[DH:CW, DH:CW] = W[h1].T
            return out

        wpack = np.concatenate(
            [
                blockdiag_t(Wq),
                blockdiag_t(Wk),
                blockdiag_t(Wv),
                np.ascontiguousarray(Wo[:, c0 : c0 + CW].T),
            ],
            axis=1,
        )
        in_maps.append(
            {
                "QTd": np.ascontiguousarray(Q[:, c0 : c0 + CW].T).astype(bf),
                "KTd": np.ascontiguousarray(K[:, c0 : c0 + CW].T).astype(bf),
                "VTd": np.ascontiguousarray(V[:, c0 : c0 + CW].T).astype(bf),
                "WPH": wpack.astype(bf),
                "Bq": np.concatenate([bq[h0], bq[h1]]).reshape(CW, 1).astype(np.float32),
                "Bvb": np.concatenate([bv[h0], bv[h1]]).reshape(1, CW).astype(np.float32),
                "WO1": np.ascontiguousarray(Wo[:, c0 + DH : c0 + CW].T).astype(bf),
            }
        )
    return in_maps


def kernel(Q, K, V, Wq, bq, Wk, bk, Wv, bv, Wo, bo, _spmd_kwargs=None):
    Q, K, V = (np.asarray(x, np.float32) for x in (Q, K, V))
    Wq, bq, Wk, bk, Wv, bv = (
        np.asarray(x, np.float32) for x in (Wq, bq, Wk, bk, Wv, bv)
    )
    Wo, bo = np.asarray(Wo, np.float32), np.asarray(bo, np.float32)

    nc = _get_nc()
    in_maps = make_in_maps(Q, K, V, Wq, bq, Wk, bk, Wv, bv, Wo)
    res = run_bass_kernel_spmd(
        nc, in_maps, core_ids=list(range(NCORES)), **(_spmd_kwargs or {})
    )

    # unshard: sum the row-parallel partials, add bo.  PTB (head-1, second
    # s-half) comes back unnormalized; divide by its denominator row DN.
    acc = np.zeros((DK, S), np.float64)
    for i in range(NCORES):
        r = res.results[i]
        acc += r["PT"].astype(np.float64)
        acc[:, S // 2 :] += r["PTB"].astype(np.float64) / r["DN"][0].astype(np.float64)[None, :]
    out = (acc.T + bo).astype(np.float32)
    if _spmd_kwargs:
        return out, res
    return out


# revision 15
# speedup vs baseline: 1.4279x; 1.0267x over previous
"""Multi-head attention (16 heads, d_model=1024, S=2048) on 8 Trainium2 cores.

Sharding: tensor-parallel over heads — each core owns 2 heads (its slice of
Wq/Wk/Wv and the matching 128 columns of Q/K/V and of Wo).  Each core computes
its heads' attention and a row-parallel partial of the final linear; the host
sums the 8 partials and adds bo (the unshard step for row-parallel).

v4 design notes:
  - Everything the PE touches is BF16 (inputs pre-transposed AND pre-cast on
    the host): the bf16 moving-operand path streams faster than fp32/f32r and
    FWL applies to the weight loads.  PSUM stays fp32 (hardware requirement).
  - Q/K/V arrive in T-space [feature, seq]; no PE transposes anywhere.
  - v is projected directly into per-chunk [t, e] attn@V stationaries
    [v0 | 1 | v1 | 1] (ones columns accumulate softmax denominators); the 16
    small v-proj matmuls are WOVEN into round 1's PE stream (pre_extras).
  - bk is dropped (softmax over t is invariant to it); bv folds into the v
    drain; bq stays.
  - exp: ACT does most chunks (fp32 scores from PSUM -> bf16); 3-4 chunks per
    round are offloaded to the DVE as a one-op Schraudolph (bf16 exp bits =
    scores * SCH_A + SCH_B computed as uint16), +-2.9% on those weights only.
  - Final linear: 32 K=128 quarters for the s-range whose normalizes finish
    early (woven into rounds 3/4), and for the last round's half: head0's
    K=64 part is woven into round 4 (PT cols 1024:2048), head1's K=64 part is
    exported UNNORMALIZED as PTB with the fp32 denominator row DN — the host
    divides during the reduce.  No normalize chain in the tail at all.
"""

import os
import sys

for _p in ("/opt/trn_rl_repo",):
    if _p not in sys.path:
        sys.path.insert(0, _p)

from contextlib import ExitStack

import ml_dtypes
import numpy as np

import concourse.bass as bass
import concourse.tile as tile
from concourse import bacc, mybir
from concourse.bass import ts
from concourse.bass_utils import run_bass_kernel_spmd

S = 2048          # sequence length
DK = 1024         # d_model
H = 16            # heads
DH = 64           # head dim
NCORES = 8
CW = 128          # per-core feature slice width (2 heads x 64)
NCH = S // 128    # 16 chunks of 128 along the sequence
VSTRIDE = 2 * (DH + 1)  # 130: [v0 64 | ones | v1 64 | ones] per chunk

F32 = mybir.dt.float32
BF16 = mybir.dt.bfloat16
U16 = mybir.dt.uint16
EXP = mybir.ActivationFunctionType.Exp

# bf16-bits Schraudolph exp for the DVE-offloaded chunks:
#   bits16(exp(0.125*s)) ~= s * SCH_A + SCH_B  (max rel err ~2.9%)
SCH_A = float(np.float32(0.125 * np.log2(np.e) * 2**7))
SCH_B = float(np.float32((127 - 0.0436775) * 2**7))
# chunks per round whose exp runs on the DVE instead of ACT
OFFLOAD = {0: (5, 11), 1: (5, 11), 2: (2, 5, 8, 11, 14), 3: (2, 5, 8, 11, 14)}

N_WARMUP_MM = 4  # PE warmup matmuls at t=0

_CACHE = {}


def _build_nc():
    nc = bacc.Bacc(
        "TRN2", target_bir_lowering=False, debug=False, enable_asserts=False
    )

    QTd = nc.dram_tensor("QTd", [CW, S], BF16, kind="ExternalInput")
    KTd = nc.dram_tensor("KTd", [CW, S], BF16, kind="ExternalInput")
    VTd = nc.dram_tensor("VTd", [CW, S], BF16, kind="ExternalInput")
    # packed bf16 weights: [wtq 128 | wtk 128 | wtv 128 | wo 1024]
    WPH = nc.dram_tensor("WPH", [CW, 3 * CW + DK], BF16, kind="ExternalInput")
    Bq = nc.dram_tensor("Bq", [CW, 1], F32, kind="ExternalInput")
    Bvb = nc.dram_tensor("Bvb", [1, CW], F32, kind="ExternalInput")
    WO1 = nc.dram_tensor("WO1", [DH, DK], BF16, kind="ExternalInput")
    PT = nc.dram_tensor("PT", [DK, S], BF16, kind="ExternalOutput")
    # head-1 contribution to the last s-half, unnormalized; host divides by DN
    PTB = nc.dram_tensor("PTB", [DK, S // 2], BF16, kind="ExternalOutput")
    DN = nc.dram_tensor("DN", [1, S // 2], F32, kind="ExternalOutput")

    with tile.TileContext(nc) as tc:
        with ExitStack() as ctx:
            pers = ctx.enter_context(tc.tile_pool(name="pers", bufs=1))
            expool = ctx.enter_context(tc.tile_pool(name="expool", bufs=3))
            stage = ctx.enter_context(tc.tile_pool(name="stage", bufs=4))
            nrm = ctx.enter_context(tc.tile_pool(name="nrm", bufs=2))
            psmm = ctx.enter_context(tc.tile_pool(name="psmm", bufs=2, space="PSUM"))
            psacc = ctx.enter_context(tc.tile_pool(name="psacc", bufs=2, space="PSUM"))
            dscr = ctx.enter_context(tc.tile_pool(name="dscr", bufs=2, space="DRAM"))

            # ---- t=0: PE warmup fodder + ACT exp table preload ----
            warm = pers.tile([128, 512], BF16, tag="warm")
            nc.gpsimd.memset(warm[:], 0.0)
            wex = pers.tile([128, 1], F32, tag="wex")
            nc.scalar.activation(wex[:], warm[:, 0:1], EXP)

            # ---- loads: K halves first (they gate the scores stream),
            # then weights, then Q, V; the two HW-DGE engines' dynamic
            # queues run their transfers in parallel ----
            QT = pers.tile([128, S], BF16, tag="QT")
            KT = pers.tile([128, S], BF16, tag="KT")
            VT = pers.tile([128, S], BF16, tag="VT")
            wp = pers.tile([CW, 3 * CW + DK], BF16, tag="wp")
            bq_sb = pers.tile([CW, 1], F32, tag="bq")
            bvb = pers.tile([128, CW], F32, tag="bvb")
            wo1_sb = pers.tile([DH, DK], BF16, tag="wo1")

            nc.sync.dma_start(KT[:, ts(0, 1024)], KTd.ap()[:, ts(0, 1024)])
            nc.scalar.dma_start(KT[:, ts(1, 1024)], KTd.ap()[:, ts(1, 1024)])
            nc.sync.dma_start(wp[:], WPH.ap())
            nc.scalar.dma_start(bq_sb[:], Bq.ap())
            nc.scalar.dma_start(bvb[:], Bvb.ap().to_broadcast((128, CW)))
            nc.sync.dma_start(QT[:, ts(0, 1024)], QTd.ap()[:, ts(0, 1024)])
            nc.scalar.dma_start(QT[:, ts(1, 1024)], QTd.ap()[:, ts(1, 1024)])
            nc.sync.dma_start(VT[:, ts(0, 1024)], VTd.ap()[:, ts(0, 1024)])
            nc.scalar.dma_start(VT[:, ts(1, 1024)], VTd.ap()[:, ts(1, 1024)])
            nc.scalar.dma_start(wo1_sb[:], WO1.ap())
            wtq_sb = wp[:, 0:CW]
            wtk_sb = wp[:, CW : 2 * CW]
            wtv_sb = wp[:, 2 * CW : 3 * CW]
            wo_sb = wp[:, 3 * CW :]

            # ---- PE warmup stream (no readers; rotates psmm ring) ----
            for _ in range(N_WARMUP_MM):
                pw = psmm.tile([128, 512], F32, tag="ps")
                nc.tensor.matmul(pw[:], warm[:, 0:128], warm[:])

            # ---- projections: xT = blockdiag(W.T) @ XT (+ b) ----
            # k lands per-head with the other head's rows zeroed (K=128
            # scores stationaries keep full PE-array activity).
            qTs = pers.tile([128, S], BF16, tag="qTs")
            kp0 = pers.tile([128, S], BF16, tag="kp0")
            kp1 = pers.tile([128, S], BF16, tag="kp1")
            kp = [kp0, kp1]
            nc.vector.memset(kp0[:], 0.0)
            nc.vector.memset(kp1[:], 0.0)
            for sl in range(S // 512):
                pp = psmm.tile([128, 512], F32, tag="ps")
                nc.tensor.matmul(pp[:], wtk_sb, KT[:, ts(sl, 512)])
                nc.vector.tensor_copy(kp0[0:DH, ts(sl, 512)], pp[0:DH, :])
                nc.scalar.copy(kp1[DH:128, ts(sl, 512)], pp[DH:128, :])
                # q right behind k (other psum pool: drains never gate the PE)
                pq = psacc.tile([128, 512], F32, tag="acc")
                nc.tensor.matmul(pq[:], wtq_sb, QT[:, ts(sl, 512)])
                nc.vector.tensor_scalar_add(qTs[:, ts(sl, 512)], pq[:], bq_sb[:])

            # ---- v in [t, e]: per-chunk stationaries with ones columns ----
            vst = pers.tile([128, NCH * VSTRIDE], BF16, tag="vst")
            ones_cols = vst[:].rearrange(
                "p (j b c) -> p j b c", j=NCH, b=2, c=DH + 1
            )[:, :, :, DH : DH + 1]
            nc.gpsimd.memset(ones_cols, 1.0)

            def vproj(j):
                def emit():
                    pv = psmm.tile([128, 128], F32, tag="ps")
                    nc.tensor.matmul(pv[:], VT[:, ts(j, 128)], wtv_sb)
                    base = j * VSTRIDE
                    dst = vst[:, base : base + VSTRIDE].rearrange(
                        "p (b c) -> p b c", b=2, c=DH + 1
                    )[:, :, 0:DH]
                    src = pv[:].rearrange("p (b c) -> p b c", b=2, c=DH)
                    bias = bvb[:].rearrange("p (b c) -> p b c", b=2, c=DH)
                    nc.vector.tensor_add(dst, src, bias)

                return emit

            vproj(0)()
            vproj(1)()

            def vstat(h, j):
                base = j * VSTRIDE + h * (DH + 1)
                return vst[:, base : base + DH + 1]

            # ---- attention, per head, per s-half ----
            oT_all = pers.tile([128, S], BF16, tag="oT")

            def attention_round(
                h, sh, extras=(), extras_from=6, pre_extras=(), last=False
            ):
                """One (head, s-half) round, software-pipelined: MM3 for
                chunk j-1 is emitted after MM2 of chunk j.  `pre_extras` fire
                between MM2(j) and MM3(j-1) (used to weave round 1's v-proj
                two chunks ahead of its consumer); `extras` fire after
                MM3(j-1) from chunk `extras_from` (final-linear quarters)."""
                hs = h * DH
                s0 = sh * 1024
                acc = psacc.tile([128, 1024], F32, tag="acc")
                exs = [None] * NCH
                extras = list(extras)
                pre_extras = list(pre_extras)
                offload = OFFLOAD[2 * sh + h]

                def mm2(j):
                    sc = psmm.tile([128, 1024], F32, tag="ps")
                    for n in range(2):
                        nc.tensor.matmul(
                            sc[:, ts(n, 512)],
                            kp[h][:, ts(j, 128)],
                            qTs[:, s0 + n * 512 : s0 + (n + 1) * 512],
                        )
                    ex = expool.tile([128, 1024], BF16, tag="ex")
                    if j in offload:
                        nc.vector.tensor_scalar(
                            ex[:].bitcast(U16), sc[:], SCH_A, SCH_B,
                            op0=mybir.AluOpType.mult, op1=mybir.AluOpType.add,
                        )
                    else:
                        nc.scalar.activation(ex[:], sc[:], EXP, scale=0.125)
                    exs[j] = ex

                def mm3(j):
                    for n in range(2):
                        nc.tensor.matmul(
                            acc[0 : DH + 1, ts(n, 512)],
                            vstat(h, j),
                            exs[j][:, ts(n, 512)],
                            start=(j == 0),
                            stop=(j == NCH - 1),
                        )

                mm2(0)
                for j in range(1, NCH):
                    mm2(j)
                    if pre_extras:
                        pre_extras.pop(0)()
                    mm3(j - 1)
                    if j >= extras_from and extras:
                        extras.pop(0)()
                mm3(NCH - 1)
                while extras:
                    extras.pop(0)()

                # acc rows (both heads): o in 0:64, denom in row 64
                if last:
                    # bf16 copy for the PTB quarters' moving operand + fp32
                    # denominator row for the host-side divide
                    ocb = nrm.tile([DH + 1, 1024], BF16, tag="ocb")
                    nc.vector.tensor_copy(ocb[:], acc[0 : DH + 1, :])
                    dn32 = nrm.tile([1, 1024], F32, tag="dn32")
                    nc.scalar.copy(dn32[0:1, :], acc[DH : DH + 1, :])
                    nc.scalar.dma_start(DN.ap(), dn32[0:1, :])
                    return ocb
                oc = nrm.tile([DH + 1, 1024], F32, tag="oc")
                nc.vector.tensor_copy(oc[:], acc[0 : DH + 1, :])
                # spread the denominator over 64 lanes for the reciprocal,
                # via DRAM (engines can't partition-scatter); broadcast back.
                dnd = dscr.tile([1, 1024], F32, tag="dnd")
                nc.sync.dma_start(dnd[:], oc[DH : DH + 1, :])
                d16 = nrm.tile([DH, 16], F32, tag="d16")
                nc.sync.dma_start(
                    d16[:], dnd[0:1, :].rearrange("a (p f) -> (a p) f", p=DH)
                )
                r16 = nrm.tile([DH, 16], F32, tag="r16")
                nc.vector.reciprocal(r16[:], d16[:])
                rnd = dscr.tile([1, 1024], F32, tag="rnd")
                nc.sync.dma_start(
                    rnd[0:1, :].rearrange("a (p f) -> (a p) f", p=DH), r16[:]
                )
                rb = nrm.tile([DH, 1024], F32, tag="rb")
                nc.sync.dma_start(rb[:], rnd[0:1, :].to_broadcast((DH, 1024)))
                nc.vector.tensor_mul(
                    oT_all[hs : hs + DH, s0 : s0 + 1024],
                    oc[0:DH, :],
                    rb[:],
                )
                return None

            def fl_pair(lhsT, rhs0, rhs1, out_slice, tag="acc", dge=None):
                """Final-linear [128,1024] pair: two matmuls into one psum
                tile, bf16 stage casts split across DVE/ACT, one 2KB-line
                DMA."""

                def emit():
                    p = (psacc if tag == "acc" else psmm).tile(
                        [128, 1024], F32, tag=tag
                    )
                    nc.tensor.matmul(p[:, 0:512], lhsT, rhs0)
                    nc.tensor.matmul(p[:, 512:1024], lhsT, rhs1)
                    st = stage.tile([128, 1024], BF16, tag="st")
                    nc.vector.tensor_copy(st[:, 0:512], p[:, 0:512])
                    nc.scalar.copy(st[:, 512:1024], p[:, 512:1024])
                    (dge or nc.sync).dma_start(out_slice, st[:])

                return emit

            # pairs for s-half 0 (both heads, K=128): woven into round 3
            fl_a = [
                fl_pair(
                    wo_sb[:, ts(mi, 128)],
                    oT_all[:, ts(0, 512)],
                    oT_all[:, ts(1, 512)],
                    PT.ap()[ts(mi, 128), 0:1024],
                )
                for mi in range(DK // 128)
            ]
            # head0's K=64 part of s-half 1 (needs round 3's normalize):
            # woven into round 4; completes PT cols 1024:2048
            fl_b = [
                fl_pair(
                    wo_sb[0:DH, ts(mi, 128)],
                    oT_all[0:DH, ts(2, 512)],
                    oT_all[0:DH, ts(3, 512)],
                    PT.ap()[ts(mi, 128), 1024:2048],
                )
                for mi in range(DK // 128)
            ]

            attention_round(0, 0, pre_extras=[vproj(j) for j in range(2, NCH)])
            attention_round(1, 0)
            attention_round(0, 1, extras=fl_a, extras_from=6)
            ocb = attention_round(1, 1, extras=fl_b, extras_from=5, last=True)

            # a few dummies bridge the ocb copy so the PE stays warm
            for _ in range(10):
                pw = psmm.tile([128, 512], F32, tag="ps")
                nc.tensor.matmul(pw[:], warm[:, 0:128], warm[:])

            # tail: head1's unnormalized K=64 pairs (host divides by DN)
            for mi in range(DK // 128):
                fl_pair(
                    wo1_sb[:, ts(mi, 128)],
                    ocb[0:DH, ts(0, 512)],
                    ocb[0:DH, ts(1, 512)],
                    PTB.ap()[ts(mi, 128), 0:1024],
                    tag="ps" if mi % 2 == 0 else "acc",
                    dge=nc.sync if mi % 2 == 0 else nc.scalar,
                )()

    nc.compile()
    return nc


def _get_nc():
    if "nc" not in _CACHE:
        _CACHE["nc"] = _build_nc()
    return _CACHE["nc"]


def make_in_maps(Q, K, V, Wq, bq, Wk, bk, Wv, bv, Wo):
    bf = ml_dtypes.bfloat16
    in_maps = []
    for i in range(NCORES):
        c0 = i * CW
        h0, h1 = 2 * i, 2 * i + 1

        def blockdiag_t(W):
            out = np.zeros((CW, CW), np.float32)
            out[0:DH, 0:DH] = W[h0].T
            out[DH:CW, DH:CW] = W[h1].T
            return out

        wpack = np.concatenate(
            [
                blockdiag_t(Wq),
                blockdiag_t(Wk),
                blockdiag_t(Wv),
                np.ascontiguousarray(Wo[:, c0 : c0 + CW].T),
            ],
            axis=1,
        )
        in_maps.append(
            {
                "QTd": np.ascontiguousarray(Q[:, c0 : c0 + CW].T).astype(bf),
                "KTd": np.ascontiguousarray(K[:, c0 : c0 + CW].T).astype(bf),
                "VTd": np.ascontiguousarray(V[:, c0 : c0 + CW].T).astype(bf),
                "WPH": wpack.astype(bf),
                "Bq": np.concatenate([bq[h0], bq[h1]]).reshape(CW, 1).astype(np.float32),
                "Bvb": np.concatenate([bv[h0], bv[h1]]).reshape(1, CW).astype(np.float32),
                "WO1": np.ascontiguousarray(Wo[:, c0 + DH : c0 + CW].T).astype(bf),
            }
        )
    return in_maps


def kernel(Q, K, V, Wq, bq, Wk, bk, Wv, bv, Wo, bo, _spmd_kwargs=None):
    Q, K, V = (np.asarray(x, np.float32) for x in (Q, K, V))
    Wq, bq, Wk, bk, Wv, bv = (
        np.asarray(x, np.float32) for x in (Wq, bq, Wk, bk, Wv, bv)
    )
    Wo, bo = np.asarray(Wo, np.float32), np.asarray(bo, np.float32)

    nc = _get_nc()
    in_maps = make_in_maps(Q, K, V, Wq, bq, Wk, bk, Wv, bv, Wo)
    res = run_bass_kernel_spmd(
        nc, in_maps, core_ids=list(range(NCORES)), **(_spmd_kwargs or {})
    )

    # unshard: sum the row-parallel partials, add bo.  PTB (head-1, second
    # s-half) comes back unnormalized; divide by its denominator row DN.
    acc = np.zeros((DK, S), np.float64)
    for i in range(NCORES):
        r = res.results[i]
        acc += r["PT"].astype(np.float64)
        acc[:, S // 2 :] += r["PTB"].astype(np.float64) / r["DN"][0].astype(np.float64)[None, :]
    out = (acc.T + bo).astype(np.float32)
    if _spmd_kwargs:
        return out, res
    return out
